# revision 12
# baseline (speedup 1.0000x reference)
"""Trainium2 Bass kernel for MultiHeadSelfAttention (GroupNorm + QKV + attention + proj + residual).

Problem shape (hardcoded): x [8, 512, 32, 32] fp32, 8 heads, 32 groups.
Sharding: data-parallel over batch B=8 across the 8 NeuronCores (one batch per core).

Per-core pipeline (T = 1024 positions, C = 512 channels, ch = 64 per head):
  1. GroupNorm(32) chunk-pipelined: groups never cross a 128-channel chunk;
     rsqrt(var+eps) via the quake bit-hack + 2 Newton steps on DVE so the
     Activation engine runs softmax exps only (no act-table swaps).
  2. qkv = qkv_w @ h with host-reordered bf16 weights:
       - q,k tiles [128, T]: m-tile 2p = [k_h(2p)|k_h(2p+1)], 2p+1 = [q...]
       - v produced transposed per s-tile, packed as fp8e4 pairs for DoubleRow
  3. Per head: logits via PE (bf16), ONE merged exp [128,1024] per s-tile on
     ACT writing fp8e4 straight to SBUF; attention @ V via fp8 DoubleRow
     matmuls (two s-planes per instruction; 64-col rider block carries the
     softmax denominator in partition 0); DVE reciprocal + gpsimd
     partition_broadcast + DVE mul to normalize.
  4. proj accumulated in PSUM per pair-group; v-bias folded into the proj
     bias on the host; bias+residual fused via scalar_tensor_tensor. Only
     the head-7 contraction half remains for the tail.

All input DMAs ride one ordered SP queue (x + wqk first) so the first
softmax exp lands as early as possible; the exp stream is the critical
resource and runs back-to-back for the rest of the kernel.
"""

import ml_dtypes
import numpy as np

import concourse.bass as bass
import concourse.bacc as bacc
import concourse.tile as tile
import concourse.mybir as mybir
from concourse import library_config
from concourse.bass_utils import run_bass_kernel_spmd

B, C, HS, WS = 8, 512, 32, 32
T = HS * WS            # 1024
H = 8                  # heads
CH = C // H            # 64
G = 32                 # groups
CPG = C // G           # 16 channels per group
EPS = 1e-5
NCHUNK = C // 128      # 4 channel chunks
NT = T // 128          # 8 sequence tiles
NB = T // 512          # 2 psum banks over T
NJ = NT // 2           # 4 s-tile pairs (DoubleRow planes)
RID = 64               # rider cols per head (col 0 = ones); out partitions 128
MAGIC = 0x5F3759DF     # quake rsqrt seed
F32 = mybir.dt.float32
F32R = mybir.dt.float32r
I32 = mybir.dt.int32
BF16 = mybir.dt.bfloat16
FP8 = mybir.dt.float8e4
EXP = mybir.ActivationFunctionType.Exp
IDENT = mybir.ActivationFunctionType.Identity
DR = mybir.MatmulPerfMode.DoubleRow
MUL = mybir.AluOpType.mult
ADD = mybir.AluOpType.add
SHR = mybir.AluOpType.logical_shift_right

_CACHE = {}


def _orig_row(kind, h, i):
    off = {"q": 0, "k": CH, "v": 2 * CH}[kind]
    return 192 * h + off + i


def _host_weights(gn_w, gn_b, qkv_w, qkv_b, proj_w, proj_b):
    scale2 = 1.0 / np.sqrt(CH)  # ch**-0.25 on both q and k -> fold into k
    rows = np.zeros(2 * C, dtype=np.int64)
    colscale = np.ones(2 * C, dtype=np.float32)
    for p in range(H // 2):
        for slot in range(2):
            h = 2 * p + slot
            for i in range(CH):
                col_k = (2 * p) * 128 + slot * CH + i
                rows[col_k] = _orig_row("k", h, i)
                colscale[col_k] = scale2
                col_q = (2 * p + 1) * 128 + slot * CH + i
                rows[col_q] = _orig_row("q", h, i)
    wqk = (qkv_w[rows, :] * colscale[:, None]).T.copy()      # [512, 1024]
    # two DMA tiles: chunks (0,1) and (2,3) side by side
    wqk_t = np.ascontiguousarray(
        wqk.reshape(2, 2, 128, 2 * C).transpose(0, 2, 1, 3).reshape(
            2, 128, 4 * C)).astype(ml_dtypes.bfloat16)
    bqk = (qkv_b[rows] * colscale).reshape(8, 128).T.copy()  # [128, 8]

    vrows = np.array([_orig_row("v", h, i) for h in range(H) for i in range(CH)])
    wv = qkv_w[vrows, :].T.copy()                            # [512, 512] (c, c_v)
    wv_t = np.ascontiguousarray(
        wv.reshape(NCHUNK, 128, C).transpose(1, 0, 2).reshape(
            128, NCHUNK * C)).astype(ml_dtypes.bfloat16)     # [128, 2048]

    bv = qkv_b[vrows]
    bproj_full = proj_b + proj_w @ bv                        # [512]
    wproj = proj_w.T.copy()                                  # [512(c), 512(o)]
    wproj_t = np.ascontiguousarray(
        wproj.reshape(NCHUNK, 128, C).transpose(1, 0, 2).reshape(
            128, NCHUNK * C)).astype(ml_dtypes.bfloat16)

    # consolidated f32 consts [128, 24]: g8 | gnw | gnb | bqk | bproj
    g8 = np.zeros((128, 8), dtype=np.float32)
    gt8 = np.zeros((8, 128), dtype=np.float32)
    for u in range(128):
        g8[u, u // CPG] = 1.0 / CPG
        gt8[u // CPG, u] = 1.0
    cst = np.concatenate([
        g8,
        gn_w.reshape(NCHUNK, 128).T,
        gn_b.reshape(NCHUNK, 128).T,
        bqk,
        bproj_full.reshape(NCHUNK, 128).T,
    ], axis=1).astype(np.float32)                            # [128, 28]
    return {"cst": cst, "gt8": gt8, "wqk": wqk_t, "wv": wv_t,
            "wproj": wproj_t}


def _build_program(n_reps=1, ew_bufs=12):
    nc = bacc.Bacc("TRN2", target_bir_lowering=False, debug=False, num_devices=8)
    dt_in = [
        ("x", [C, T], F32), ("cst", [128, 28], F32R), ("gt8", [8, 128], F32R),
        ("wqk", [2, 128, 4 * C], BF16), ("wv", [128, NCHUNK * C], BF16),
        ("wproj", [128, NCHUNK * C], BF16),
    ]
    d = {name: nc.dram_tensor(name, shape, dt, kind="ExternalInput").ap()
         for name, shape, dt in dt_in}
    out_d = nc.dram_tensor("out", [C, T], F32, kind="ExternalOutput").ap()

    with tile.TileContext(nc) as tc:
        with (
            tc.tile_pool(name="singles", bufs=1) as singles,
            tc.tile_pool(name="small", bufs=16) as small,
            tc.tile_pool(name="ewp", bufs=ew_bufs) as ewp,
            tc.tile_pool(name="recp", bufs=2) as recp,
            tc.tile_pool(name="rbp", bufs=2) as rbp,
            tc.tile_pool(name="psA", bufs=2, space="PSUM") as psA,
            tc.tile_pool(name="psB", bufs=2, space="PSUM") as psB,
        ):
            nc.gpsimd.load_library(library_config.attn)

            # ---- one ordered DMA stream on the SP queue: consts, then x
            # ---- halves interleaved with wqk, then wv/wproj ----
            cst = singles.tile([128, 28], F32R, tag="cst", name="cst")
            nc.sync.dma_start(cst[:], d["cst"][:])
            gt8_sb = singles.tile([8, 128], F32R, tag="gt8", name="gt8")
            nc.sync.dma_start(gt8_sb[:], d["gt8"][:])
            g8_sb = cst[:, 0:8]
            gnw_sb = cst[:, 8:12].bitcast(F32)
            gnb_sb = cst[:, 12:16].bitcast(F32)
            bqk_sb = cst[:, 16:24].bitcast(F32)
            bproj_sb = cst[:, 24:28].bitcast(F32)

            x_sb = [singles.tile([128, T], F32, tag=f"x{k}", name=f"x{k}")
                    for k in range(NCHUNK)]
            wqk_sb = [singles.tile([128, 4 * C], BF16, tag=f"wqk{g}",
                                   name=f"wqk{g}") for g in range(2)]
            for k in range(NCHUNK):
                for nb in range(NB):
                    sl = slice(512 * nb, 512 * (nb + 1))
                    nc.sync.dma_start(x_sb[k][:, sl],
                                      d["x"][128 * k:128 * (k + 1), sl])
                if k == 1:
                    nc.sync.dma_start(wqk_sb[0][:], d["wqk"][0])
                if k == 3:
                    nc.sync.dma_start(wqk_sb[1][:], d["wqk"][1])
            wv_sb = singles.tile([128, NCHUNK * C], BF16, tag="wv", name="wv")
            nc.sync.dma_start(wv_sb[:], d["wv"][:])
            wproj_sb = singles.tile([128, NCHUNK * C], BF16, tag="wproj",
                                    name="wproj")
            nc.sync.dma_start(wproj_sb[:], d["wproj"][:])

            def wqk_ap(k, m):
                # chunk k, m-tile column block [128, 128]
                return wqk_sb[k // 2][:, 1024 * (k % 2) + 128 * m:
                                      1024 * (k % 2) + 128 * (m + 1)]

            def wv_ap(k):
                return wv_sb[:, 512 * k:512 * (k + 1)]

            def wproj_ap(p, m, clo=0, chi=128):
                return wproj_sb[clo:chi, 512 * p + 128 * m:512 * p + 128 * (m + 1)]

            magic_t = singles.tile([8, 1], I32, tag="magic", name="magic")
            nc.vector.memset(magic_t[:], MAGIC)

            for rep in range(n_reps):
                sfx = f"r{rep}"
                # ================= GroupNorm (per chunk) =================
                h_sb = []
                for k in range(NCHUNK):
                    stats = small.tile([128, 2], F32R, tag="small", name="stats")
                    if k < 2:
                        # per-channel sum / sum-of-squares via ACT accumulators
                        scr = small.tile([128, T], BF16, tag="gnscr", bufs=2,
                                         name="scr")
                        asm = small.tile([128, 1], F32, tag="small", name="asm")
                        nc.scalar.activation(scr[:], x_sb[k][:], IDENT,
                                             accum_out=asm[:])
                        asq = small.tile([128, 1], F32, tag="small", name="asq")
                        nc.scalar.activation(
                            scr[:], x_sb[k][:],
                            mybir.ActivationFunctionType.Square,
                            accum_out=asq[:])
                        nc.vector.tensor_scalar(out=stats[:, 0:1], in0=asm[:],
                                                scalar1=1.0 / T, scalar2=None,
                                                op0=MUL)
                        nc.vector.tensor_scalar(out=stats[:, 1:2], in0=asq[:],
                                                scalar1=1.0 / T, scalar2=None,
                                                op0=MUL)
                    else:
                        st6 = small.tile([128, 2, 6], F32, tag="small",
                                         name="st6")
                        nc.vector.bn_stats(st6[:, 0, :], x_sb[k][:, 0:512])
                        nc.vector.bn_stats(st6[:, 1, :], x_sb[k][:, 512:1024])
                        mv = small.tile([128, 2], F32, tag="small", name="mv")
                        nc.vector.bn_aggr(mv[:], st6[:])
                        m2 = small.tile([128, 1], F32, tag="small", name="m2")
                        nc.vector.tensor_mul(m2[:], mv[:, 0:1], mv[:, 0:1])
                        nc.vector.tensor_copy(stats[:, 0:1], mv[:, 0:1])
                        nc.vector.tensor_add(stats[:, 1:2], mv[:, 1:2], m2[:])
                    psg = psA.tile([8, 2], F32, tag="big", name="psg")
                    nc.tensor.matmul(psg[:], g8_sb, stats[:],
                                     start=True, stop=True)
                    gsb = small.tile([8, 2], F32, tag="small", name="gsb")
                    nc.vector.tensor_copy(gsb[:], psg[:])
                    mu2 = small.tile([8, 1], F32, tag="small", name="mu2")
                    nc.vector.tensor_mul(mu2[:], gsb[:, 0:1], gsb[:, 0:1])
                    # a = var + eps ;  rstd = rsqrt(a) via bit hack + 2 Newton
                    av = small.tile([8, 1], F32, tag="small", name="av")
                    nc.vector.tensor_sub(av[:], gsb[:, 1:2], mu2[:])
                    nc.vector.tensor_scalar(out=av[:], in0=av[:], scalar1=EPS,
                                            scalar2=None, op0=ADD)
                    yi = small.tile([8, 1], I32, tag="small", name="yi")
                    nc.vector.tensor_scalar(out=yi[:], in0=av[:].bitcast(I32),
                                            scalar1=1, scalar2=None, op0=SHR)
                    nc.vector.tensor_sub(yi[:], magic_t[:], yi[:])
                    y = yi[:].bitcast(F32)
                    ah = small.tile([8, 1], F32, tag="small", name="ah")
                    nc.vector.tensor_scalar(out=ah[:], in0=av[:], scalar1=0.5,
                                            scalar2=None, op0=MUL)
                    t2 = small.tile([8, 1], F32, tag="small", name="t2")
                    for _ in range(1):
                        nc.vector.tensor_mul(t2[:], y, y)
                        nc.vector.tensor_mul(t2[:], t2[:], ah[:])
                        nc.vector.tensor_scalar(out=t2[:], in0=t2[:],
                                                scalar1=-1.0, scalar2=1.5,
                                                op0=MUL, op1=ADD)
                        nc.vector.tensor_mul(y, y, t2[:])
                    grp = small.tile([8, 2], F32R, tag="small", name="grp")
                    nc.vector.tensor_copy(grp[:, 0:1], gsb[:, 0:1])
                    nc.vector.tensor_copy(grp[:, 1:2], y)
                    psc = psA.tile([128, 2], F32, tag="big", name="psc")
                    nc.tensor.matmul(psc[:], gt8_sb[:], grp[:],
                                     start=True, stop=True)
                    s_c = small.tile([128, 1], F32, tag="small", name="s_c")
                    nc.vector.tensor_mul(s_c[:], psc[:, 1:2], gnw_sb[:, k:k + 1])
                    t1 = small.tile([128, 1], F32, tag="small", name="t1")
                    nc.vector.tensor_mul(t1[:], psc[:, 0:1], s_c[:])
                    b_c = small.tile([128, 1], F32, tag="small", name="b_c")
                    nc.vector.tensor_sub(b_c[:], gnb_sb[:, k:k + 1], t1[:])
                    ht = singles.tile([128, T], BF16, tag=f"h{k}", name=f"h{k}")
                    for nb in range(NB):
                        sl = slice(512 * nb, 512 * (nb + 1))
                        if k == 2:
                            nc.vector.tensor_scalar(
                                out=ht[:, sl], in0=x_sb[k][:, sl],
                                scalar1=s_c[:], scalar2=b_c[:], op0=MUL,
                                op1=ADD)
                        else:
                            nc.scalar.activation(ht[:, sl], x_sb[k][:, sl],
                                                 IDENT, bias=b_c[:],
                                                 scale=s_c[:])
                    h_sb.append(ht)

                # ================= qk tiles =================
                qk_tiles = {}

                def gen_qk01():
                    # m = 0, 1 interleaved nb-major so QK(0) steps on the
                    # first t-half can start as early as possible
                    pqs = [psA.tile([128, T], F32, tag="big", name="pq")
                           for _ in range(2)]
                    for m in range(2):
                        qk_tiles[m] = singles.tile(
                            [128, T], BF16, tag=f"qk{m}{sfx}", name=f"qk{m}")
                    for nb in range(NB):
                        sl = slice(512 * nb, 512 * (nb + 1))
                        for m in range(2):
                            for k in range(NCHUNK):
                                nc.tensor.matmul(
                                    pqs[m][:, sl], wqk_ap(k, m),
                                    h_sb[k][:, sl], start=(k == 0),
                                    stop=(k == 3))
                        nc.scalar.activation(qk_tiles[0][:, sl], pqs[0][:, sl],
                                             IDENT, bias=bqk_sb[:, 0:1])
                        nc.vector.tensor_scalar(
                            out=qk_tiles[1][:, sl], in0=pqs[1][:, sl],
                            scalar1=bqk_sb[:, 1:2], scalar2=None, op0=ADD)

                gen_qk01()

                # ================= attention state =================
                ew_pairs = {}

                def _ew(h, j):
                    if (h, j) not in ew_pairs:
                        ew_pairs[(h, j)] = ewp.tile([128, 2, T], FP8, tag="ew",
                                                    name=f"ew{h}_{j}")
                    return ew_pairs[(h, j)]

                def emit_qk_step(h, st):
                    # logits for head h, s-tile st: 2 matmuls + 1 merged exp
                    p, slot = h // 2, h % 2
                    lo, hi = CH * slot, CH * (slot + 1)
                    ktile, qtile = qk_tiles[2 * p], qk_tiles[2 * p + 1]
                    j, pl = st // 2, st % 2
                    pw = psB.tile([128, T], F32, tag="pw", name="pw")
                    for nb in range(NB):
                        nc.tensor.matmul(
                            pw[:, 512 * nb:512 * (nb + 1)],
                            ktile[lo:hi, 128 * st:128 * (st + 1)],
                            qtile[lo:hi, 512 * nb:512 * (nb + 1)],
                            start=True, stop=True)
                    nc.scalar.activation(_ew(h, j)[:, pl, :], pw[:], EXP)

                def emit_qk_half_step(h, st, nb):
                    # one t-half of head h's logits (used to stretch head 7's
                    # exp stream over the last two windows)
                    p, slot = h // 2, h % 2
                    lo, hi = CH * slot, CH * (slot + 1)
                    ktile, qtile = qk_tiles[2 * p], qk_tiles[2 * p + 1]
                    j, pl = st // 2, st % 2
                    sl = slice(512 * nb, 512 * (nb + 1))
                    pw = psB.tile([128, 512], F32, tag="pw", name="pwh")
                    nc.tensor.matmul(
                        pw[:], ktile[lo:hi, 128 * st:128 * (st + 1)],
                        qtile[lo:hi, sl], start=True, stop=True)
                    nc.scalar.activation(_ew(h, j)[:, pl, sl], pw[:], EXP)

                # vt pair tiles (fp8, rider block cols 0:RID with col0 = ones)
                vt_sb = [singles.tile([128, 2, H, RID + CH], FP8,
                                      tag=f"vt{j}", name=f"vt{j}")
                         for j in range(NJ)]
                for j in range(NJ):
                    nc.vector.memset(vt_sb[j][:, :, :, 0:RID], 0.0)
                    nc.vector.memset(vt_sb[j][:, :, :, 0:1], 1.0)

                def emit_v_tile(st):
                    pv = psA.tile([128, C], F32, tag="big", name="pv")
                    for k in range(NCHUNK):
                        nc.tensor.matmul(pv[:],
                                         h_sb[k][:, 128 * st:128 * (st + 1)],
                                         wv_ap(k), start=(k == 0),
                                         stop=(k == 3))
                    nc.vector.tensor_copy(
                        vt_sb[st // 2][:, st % 2, :, RID:RID + CH],
                        pv[:].rearrange("p (h c) -> p h c", h=H))

                a_sb = [singles.tile([128, T], BF16, tag=f"a{p}",
                                     name=f"a{p}{sfx}") for p in range(NCHUNK)]
                acc_sb = [singles.tile([128, T], F32, tag=f"acc{m}",
                                       name=f"acc{m}{sfx}")
                          for m in range(NCHUNK)]

                # ---- prologue: v tiles + QK(0) steps (pv uses the big tag
                # ---- so the pw rotation stays a pure QK/exp double-buffer)
                for st in range(NT):
                    emit_qk_step(0, st)
                    emit_v_tile(st)

                # ================= duties =================
                def qk_spread_duty(m):
                    pq = psA.tile([128, T], F32, tag="big", name="pq")
                    qk = singles.tile([128, T], BF16, tag=f"qk{m}{sfx}",
                                      name=f"qk{m}")
                    qk_tiles[m] = qk
                    for nb in range(NB):
                        sl = slice(512 * nb, 512 * (nb + 1))
                        for k in range(NCHUNK):
                            nc.tensor.matmul(
                                pq[:, sl], wqk_ap(k, m), h_sb[k][:, sl],
                                start=(k == 0), stop=(k == 3))
                            yield
                        nc.vector.tensor_scalar(
                            out=qk[:, sl], in0=pq[:, sl],
                            scalar1=bqk_sb[:, m:m + 1], scalar2=None, op0=ADD)
                    yield

                def proj01_duty():
                    # acc[m] = (Wp0 @ a0 + Wp1 @ a1 + bproj) + x
                    for m in range(NCHUNK):
                        po = psA.tile([128, T], F32, tag="big", name="po")
                        for nb in range(NB):
                            sl = slice(512 * nb, 512 * (nb + 1))
                            nc.tensor.matmul(
                                po[:, sl], wproj_ap(0, m), a_sb[0][:, sl],
                                start=True, stop=False)
                            yield
                            nc.tensor.matmul(
                                po[:, sl], wproj_ap(1, m), a_sb[1][:, sl],
                                start=False, stop=True)
                            yield
                        nc.vector.scalar_tensor_tensor(
                            out=acc_sb[m][:], in0=po[:],
                            scalar=bproj_sb[:, m:m + 1], in1=x_sb[m][:],
                            op0=ADD, op1=ADD)
                        yield

                def proj2_duty(mlo, mhi):
                    # acc[m] += Wp2 @ a2
                    for m in range(mlo, mhi):
                        po = psA.tile([128, T], F32, tag="big", name="po")
                        for nb in range(NB):
                            sl = slice(512 * nb, 512 * (nb + 1))
                            nc.tensor.matmul(
                                po[:, sl], wproj_ap(2, m), a_sb[2][:, sl],
                                start=True, stop=True)
                            yield
                        nc.vector.tensor_add(acc_sb[m][:], po[:], acc_sb[m][:])
                        yield

                # ================= head loop =================
                last_rep = rep == n_reps - 1
                for h in range(H):
                    p, slot = h // 2, h % 2
                    lo, hi = CH * slot, CH * (slot + 1)
                    duties = []
                    if h == 0:
                        duties.append(qk_spread_duty(2))
                        duties.append(qk_spread_duty(3))
                    elif h == 1:
                        duties.append(qk_spread_duty(4))
                    elif h == 2:
                        duties.append(qk_spread_duty(5))
                    elif h == 3:
                        duties.append(qk_spread_duty(6))
                    elif h == 4:
                        duties.append(qk_spread_duty(7))
                    elif h == 5:
                        duties.append(proj01_duty())
                    elif h == 6:
                        duties.append(proj2_duty(0, 2))
                    pa = psA.tile([128, T], F32, tag="big", name="pa")

                    def tail_nb(nb, pa=pa):
                        # normalize head 7's nb half, then pair-3 proj + out
                        sl = slice(512 * nb, 512 * (nb + 1))
                        rcb = recp.tile([1, 512], F32, tag="rcb", name="rcb")
                        nc.vector.reciprocal_approx_fast(rcb[:], pa[0:1, sl])
                        rbb = rbp.tile([CH, 512], F32, tag="rbb", name="rbb")
                        nc.gpsimd.partition_broadcast(rbb[:], rcb[:])
                        nc.vector.tensor_mul(
                            a_sb[3][CH:128, sl], pa[RID:RID + CH, sl], rbb[:])
                        yield
                        for m in range(NCHUNK):
                            po = psA.tile([128, 512], F32, tag="big",
                                          name="pot")
                            nc.tensor.matmul(po[:], wproj_ap(3, m),
                                             a_sb[3][:, sl],
                                             start=True, stop=True)
                            nc.vector.tensor_add(acc_sb[m][:, sl], po[:],
                                                 acc_sb[m][:, sl])
                            if last_rep:
                                nc.sync.dma_start(
                                    out_d[128 * m:128 * (m + 1), sl],
                                    acc_sb[m][:, sl])
                            yield

                    if h == 7:
                        duties.append(proj2_duty(2, 4))
                        # nb0 logits were made during window 6: all nb0 AV now
                        for j in range(NJ):
                            nc.tensor.matmul(
                                pa[:, 0:512], vt_sb[j][:, :, 7, :],
                                ew_pairs[(7, j)][:, :, 0:512],
                                start=(j == 0), stop=(j == NJ - 1),
                                perf_mode=DR)
                        duties.append(tail_nb(0))
                    else:
                        # ew for this head is complete: AV burst at window
                        # start, normalization runs mid-window
                        for j in range(NJ):
                            ewt = ew_pairs.pop((h, j))
                            for nb in range(NB):
                                nc.tensor.matmul(
                                    pa[:, 512 * nb:512 * (nb + 1)],
                                    vt_sb[j][:, :, h, :],
                                    ewt[:, :, 512 * nb:512 * (nb + 1)],
                                    start=(j == 0), stop=(j == NJ - 1),
                                    perf_mode=DR)
                        rec = recp.tile([1, T], F32, tag="rec", name="rec")
                        nc.vector.reciprocal_approx_fast(rec[:], pa[0:1, :])
                        rb = rbp.tile([CH, T], F32, tag="rb", name="rb")
                        nc.gpsimd.partition_broadcast(rb[:], rec[:])
                        nc.vector.tensor_mul(a_sb[p][lo:hi, :],
                                             pa[RID:RID + CH, :], rb[:])
                    for st in range(NT):
                        if h < 6:
                            emit_qk_step(h + 1, st)
                        elif h == 6:
                            emit_qk_half_step(7, st, 0)
                        else:
                            emit_qk_half_step(7, st, 1)
                        advanced = 0
                        while duties and advanced < 3:
                            try:
                                next(duties[0])
                                advanced += 1
                            except StopIteration:
                                duties.pop(0)
                        if h == 7 and st % 2 == 1:
                            j = st // 2
                            ewt = ew_pairs.pop((7, j))
                            nc.tensor.matmul(
                                pa[:, 512:1024], vt_sb[j][:, :, 7, :],
                                ewt[:, :, 512:1024],
                                start=(j == 0), stop=(j == NJ - 1),
                                perf_mode=DR)
                    for g in duties:
                        for _ in g:
                            pass
                    if h == 7:
                        for _ in tail_nb(1):
                            pass

    nc.compile()
    return nc


def _get_program(n_reps=1):
    key = ("prog", n_reps)
    if key not in _CACHE:
        _CACHE[key] = _build_program(n_reps)
    return _CACHE[key]


def kernel(x, gn_w, gn_b, qkv_w, qkv_b, proj_w, proj_b, _n_reps=1):
    x = np.asarray(x, dtype=np.float32)
    hw = _host_weights(np.asarray(gn_w, np.float32), np.asarray(gn_b, np.float32),
                       np.asarray(qkv_w, np.float32), np.asarray(qkv_b, np.float32),
                       np.asarray(proj_w, np.float32), np.asarray(proj_b, np.float32))
    xr = np.ascontiguousarray(x.reshape(B, C, T))
    nc = _get_program(_n_reps)
    in_maps = [dict(hw, x=xr[b]) for b in range(B)]
    res = run_bass_kernel_spmd(nc, in_maps, core_ids=list(range(B)))
    out = np.stack([res.results[b]["out"] for b in range(B)])
    return out.reshape(B, C, HS, WS).astype(np.float32)


# revision 13
# speedup vs baseline: 1.0456x; 1.0456x over previous
"""Trainium2 Bass kernel for MultiHeadSelfAttention (GroupNorm + QKV + attention + proj + residual).

Problem shape (hardcoded): x [8, 512, 32, 32] fp32, 8 heads, 32 groups.
Sharding: data-parallel over batch B=8 across the 8 NeuronCores (one batch per core).

Per-core pipeline (T = 1024 positions, C = 512 channels, ch = 64 per head):
  1. GroupNorm(32) chunk-pipelined: groups never cross a 128-channel chunk;
     rsqrt(var+eps) via the quake bit-hack + 2 Newton steps on DVE so the
     Activation engine runs softmax exps only (no act-table swaps).
  2. qkv = qkv_w @ h with host-reordered bf16 weights:
       - q,k tiles [128, T]: m-tile 2p = [k_h(2p)|k_h(2p+1)], 2p+1 = [q...]
       - v produced transposed per s-tile, packed as fp8e4 pairs for DoubleRow
  3. Per head: logits via PE (bf16), ONE merged exp [128,1024] per s-tile on
     ACT writing fp8e4 straight to SBUF; attention @ V via fp8 DoubleRow
     matmuls (two s-planes per instruction; 64-col rider block carries the
     softmax denominator in partition 0); DVE reciprocal + gpsimd
     partition_broadcast + DVE mul to normalize.
  4. proj accumulated in PSUM per pair-group; v-bias folded into the proj
     bias on the host; bias+residual fused via scalar_tensor_tensor. Only
     the head-7 contraction half remains for the tail.

All input DMAs ride one ordered SP queue (x + wqk first) so the first
softmax exp lands as early as possible; the exp stream is the critical
resource and runs back-to-back for the rest of the kernel.
"""

import ml_dtypes
import numpy as np

import concourse.bass as bass
import concourse.bacc as bacc
import concourse.tile as tile
import concourse.mybir as mybir
from concourse import library_config
from concourse.bass_utils import run_bass_kernel_spmd

B, C, HS, WS = 8, 512, 32, 32
T = HS * WS            # 1024
H = 8                  # heads
CH = C // H            # 64
G = 32                 # groups
CPG = C // G           # 16 channels per group
EPS = 1e-5
NCHUNK = C // 128      # 4 channel chunks
NT = T // 128          # 8 sequence tiles
NB = T // 512          # 2 psum banks over T
NJ = NT // 2           # 4 s-tile pairs (DoubleRow planes)
RID = 64               # rider cols per head (col 0 = ones); out partitions 128
MAGIC = 0x5F3759DF     # quake rsqrt seed
F32 = mybir.dt.float32
F32R = mybir.dt.float32r
I32 = mybir.dt.int32
BF16 = mybir.dt.bfloat16
FP8 = mybir.dt.float8e4
EXP = mybir.ActivationFunctionType.Exp
IDENT = mybir.ActivationFunctionType.Identity
DR = mybir.MatmulPerfMode.DoubleRow
MUL = mybir.AluOpType.mult
ADD = mybir.AluOpType.add
SHR = mybir.AluOpType.logical_shift_right

_CACHE = {}


def _orig_row(kind, h, i):
    off = {"q": 0, "k": CH, "v": 2 * CH}[kind]
    return 192 * h + off + i


def _host_weights(gn_w, gn_b, qkv_w, qkv_b, proj_w, proj_b):
    scale2 = 1.0 / np.sqrt(CH)  # ch**-0.25 on both q and k -> fold into k
    rows = np.zeros(2 * C, dtype=np.int64)
    colscale = np.ones(2 * C, dtype=np.float32)
    for p in range(H // 2):
        for slot in range(2):
            h = 2 * p + slot
            for i in range(CH):
                col_k = (2 * p) * 128 + slot * CH + i
                rows[col_k] = _orig_row("k", h, i)
                colscale[col_k] = scale2
                col_q = (2 * p + 1) * 128 + slot * CH + i
                rows[col_q] = _orig_row("q", h, i)
    wqk = (qkv_w[rows, :] * colscale[:, None]).T.copy()      # [512, 1024]
    # two DMA tiles: chunks (0,1) and (2,3) side by side
    wqk_t = np.ascontiguousarray(
        wqk.reshape(2, 2, 128, 2 * C).transpose(0, 2, 1, 3).reshape(
            2, 128, 4 * C)).astype(ml_dtypes.bfloat16)
    bqk = (qkv_b[rows] * colscale).reshape(8, 128).T.copy()  # [128, 8]

    vrows = np.array([_orig_row("v", h, i) for h in range(H) for i in range(CH)])
    wv = qkv_w[vrows, :].T.copy()                            # [512, 512] (c, c_v)
    wv_t = np.ascontiguousarray(
        wv.reshape(NCHUNK, 128, C).transpose(1, 0, 2).reshape(
            128, NCHUNK * C)).astype(ml_dtypes.bfloat16)     # [128, 2048]

    bv = qkv_b[vrows]
    bproj_full = proj_b + proj_w @ bv                        # [512]
    wproj = proj_w.T.copy()                                  # [512(c), 512(o)]
    wproj_t = np.ascontiguousarray(
        wproj.reshape(NCHUNK, 128, C).transpose(1, 0, 2).reshape(
            128, NCHUNK * C)).astype(ml_dtypes.bfloat16)

    # consolidated f32 consts [128, 24]: g8 | gnw | gnb | bqk | bproj
    g8 = np.zeros((128, 8), dtype=np.float32)
    gt8 = np.zeros((8, 128), dtype=np.float32)
    for u in range(128):
        g8[u, u // CPG] = 1.0 / CPG
        gt8[u // CPG, u] = 1.0
    cst = np.concatenate([
        g8,
        gn_w.reshape(NCHUNK, 128).T,
        gn_b.reshape(NCHUNK, 128).T,
        bqk,
        bproj_full.reshape(NCHUNK, 128).T,
    ], axis=1).astype(np.float32)                            # [128, 28]
    return {"cst": cst, "gt8": gt8, "wqk": wqk_t, "wv": wv_t,
            "wproj": wproj_t}


def _build_program(n_reps=1, ew_bufs=12):
    nc = bacc.Bacc("TRN2", target_bir_lowering=False, debug=False, num_devices=8)
    dt_in = [
        ("x", [C, T], F32), ("cst", [128, 28], F32R), ("gt8", [8, 128], F32R),
        ("wqk", [2, 128, 4 * C], BF16), ("wv", [128, NCHUNK * C], BF16),
        ("wproj", [128, NCHUNK * C], BF16),
    ]
    d = {name: nc.dram_tensor(name, shape, dt, kind="ExternalInput").ap()
         for name, shape, dt in dt_in}
    out_d = nc.dram_tensor("out", [C, T], F32, kind="ExternalOutput").ap()

    with tile.TileContext(nc) as tc:
        with (
            tc.tile_pool(name="singles", bufs=1) as singles,
            tc.tile_pool(name="small", bufs=16) as small,
            tc.tile_pool(name="ewp", bufs=ew_bufs) as ewp,
            tc.tile_pool(name="recp", bufs=2) as recp,
            tc.tile_pool(name="rbp", bufs=2) as rbp,
            tc.tile_pool(name="psA", bufs=2, space="PSUM") as psA,
            tc.tile_pool(name="psB", bufs=2, space="PSUM") as psB,
        ):
            nc.gpsimd.load_library(library_config.attn)

            # ---- one ordered DMA stream on the SP queue: consts, then x
            # ---- halves interleaved with wqk, then wv/wproj ----
            cst = singles.tile([128, 28], F32R, tag="cst", name="cst")
            nc.sync.dma_start(cst[:], d["cst"][:])
            gt8_sb = singles.tile([8, 128], F32R, tag="gt8", name="gt8")
            nc.sync.dma_start(gt8_sb[:], d["gt8"][:])
            g8_sb = cst[:, 0:8]
            gnw_sb = cst[:, 8:12].bitcast(F32)
            gnb_sb = cst[:, 12:16].bitcast(F32)
            bqk_sb = cst[:, 16:24].bitcast(F32)
            bproj_sb = cst[:, 24:28].bitcast(F32)

            x_sb = [singles.tile([128, T], F32, tag=f"x{k}", name=f"x{k}")
                    for k in range(NCHUNK)]
            wqk_sb = [singles.tile([128, 4 * C], BF16, tag=f"wqk{g}",
                                   name=f"wqk{g}") for g in range(2)]
            for k in range(NCHUNK):
                for nb in range(NB):
                    sl = slice(512 * nb, 512 * (nb + 1))
                    nc.sync.dma_start(x_sb[k][:, sl],
                                      d["x"][128 * k:128 * (k + 1), sl])
            for g in range(2):
                nc.sync.dma_start(wqk_sb[g][:], d["wqk"][g])
            wv_sb = singles.tile([128, NCHUNK * C], BF16, tag="wv", name="wv")
            nc.sync.dma_start(wv_sb[:], d["wv"][:])
            wproj_sb = singles.tile([128, NCHUNK * C], BF16, tag="wproj",
                                    name="wproj")
            nc.sync.dma_start(wproj_sb[:], d["wproj"][:])

            def wqk_ap(k, m):
                # chunk k, m-tile column block [128, 128]
                return wqk_sb[k // 2][:, 1024 * (k % 2) + 128 * m:
                                      1024 * (k % 2) + 128 * (m + 1)]

            def wv_ap(k):
                return wv_sb[:, 512 * k:512 * (k + 1)]

            def wproj_ap(p, m, clo=0, chi=128):
                return wproj_sb[clo:chi, 512 * p + 128 * m:512 * p + 128 * (m + 1)]

            magic_t = singles.tile([8, 1], I32, tag="magic", name="magic")
            nc.vector.memset(magic_t[:], MAGIC)
            # prime the exp/identity activation table while ACT is idle
            prim = singles.tile([1, 1], F32, tag="prim", name="prim")
            nc.vector.memset(prim[:], 0.0)
            nc.scalar.activation(prim[:], prim[:], IDENT)

            for rep in range(n_reps):
                sfx = f"r{rep}"
                # ================= GroupNorm (per chunk) =================
                h_sb = [None] * NCHUNK
                stats_t = {}
                # phase 1: per-channel stats -- chunks 2,3 on DVE first (the
                # late-arriving x tiles), chunks 0,1 via ACT accumulators
                for k in (2, 3):
                    stats = small.tile([128, 2], F32R, tag="small", name="stats")
                    st6 = small.tile([128, 2, 6], F32, tag="small", name="st6")
                    nc.vector.bn_stats(st6[:, 0, :], x_sb[k][:, 0:512])
                    nc.vector.bn_stats(st6[:, 1, :], x_sb[k][:, 512:1024])
                    mv = small.tile([128, 2], F32, tag="small", name="mv")
                    nc.vector.bn_aggr(mv[:], st6[:])
                    m2 = small.tile([128, 1], F32, tag="small", name="m2")
                    nc.vector.tensor_mul(m2[:], mv[:, 0:1], mv[:, 0:1])
                    nc.vector.tensor_copy(stats[:, 0:1], mv[:, 0:1])
                    nc.vector.tensor_add(stats[:, 1:2], mv[:, 1:2], m2[:])
                    stats_t[k] = stats
                for k in (0, 1):
                    stats = small.tile([128, 2], F32R, tag="small", name="stats")
                    scr = small.tile([128, T], BF16, tag="gnscr", bufs=2,
                                     name="scr")
                    asm = small.tile([128, 1], F32, tag="small", name="asm")
                    nc.scalar.activation(scr[:], x_sb[k][:], IDENT,
                                         accum_out=asm[:])
                    asq = small.tile([128, 1], F32, tag="small", name="asq")
                    nc.scalar.activation(scr[:], x_sb[k][:],
                                         mybir.ActivationFunctionType.Square,
                                         accum_out=asq[:])
                    nc.vector.tensor_scalar(out=stats[:, 0:1], in0=asm[:],
                                            scalar1=1.0 / T, scalar2=None,
                                            op0=MUL)
                    nc.vector.tensor_scalar(out=stats[:, 1:2], in0=asq[:],
                                            scalar1=1.0 / T, scalar2=None,
                                            op0=MUL)
                    stats_t[k] = stats
                # phase 2: group combine + affine, in stats-arrival order
                for k in (0, 2, 1, 3):
                    stats = stats_t[k]
                    psg = psA.tile([8, 2], F32, tag="big", name="psg")
                    nc.tensor.matmul(psg[:], g8_sb, stats[:],
                                     start=True, stop=True)
                    gsb = small.tile([8, 2], F32, tag="small", name="gsb")
                    nc.vector.tensor_copy(gsb[:], psg[:])
                    mu2 = small.tile([8, 1], F32, tag="small", name="mu2")
                    nc.vector.tensor_mul(mu2[:], gsb[:, 0:1], gsb[:, 0:1])
                    av = small.tile([8, 1], F32, tag="small", name="av")
                    nc.vector.tensor_sub(av[:], gsb[:, 1:2], mu2[:])
                    nc.vector.tensor_scalar(out=av[:], in0=av[:], scalar1=EPS,
                                            scalar2=None, op0=ADD)
                    yi = small.tile([8, 1], I32, tag="small", name="yi")
                    nc.vector.tensor_scalar(out=yi[:], in0=av[:].bitcast(I32),
                                            scalar1=1, scalar2=None, op0=SHR)
                    nc.vector.tensor_sub(yi[:], magic_t[:], yi[:])
                    y = yi[:].bitcast(F32)
                    ah = small.tile([8, 1], F32, tag="small", name="ah")
                    nc.vector.tensor_scalar(out=ah[:], in0=av[:], scalar1=0.5,
                                            scalar2=None, op0=MUL)
                    t2 = small.tile([8, 1], F32, tag="small", name="t2")
                    nc.vector.tensor_mul(t2[:], y, y)
                    nc.vector.tensor_mul(t2[:], t2[:], ah[:])
                    nc.vector.tensor_scalar(out=t2[:], in0=t2[:],
                                            scalar1=-1.0, scalar2=1.5,
                                            op0=MUL, op1=ADD)
                    nc.vector.tensor_mul(y, y, t2[:])
                    grp = small.tile([8, 2], F32R, tag="small", name="grp")
                    nc.vector.tensor_copy(grp[:, 0:1], gsb[:, 0:1])
                    nc.vector.tensor_copy(grp[:, 1:2], y)
                    psc = psA.tile([128, 2], F32, tag="big", name="psc")
                    nc.tensor.matmul(psc[:], gt8_sb[:], grp[:],
                                     start=True, stop=True)
                    s_c = small.tile([128, 1], F32, tag="small", name="s_c")
                    nc.vector.tensor_mul(s_c[:], psc[:, 1:2], gnw_sb[:, k:k + 1])
                    t1 = small.tile([128, 1], F32, tag="small", name="t1")
                    nc.vector.tensor_mul(t1[:], psc[:, 0:1], s_c[:])
                    b_c = small.tile([128, 1], F32, tag="small", name="b_c")
                    nc.vector.tensor_sub(b_c[:], gnb_sb[:, k:k + 1], t1[:])
                    ht = singles.tile([128, T], BF16, tag=f"h{k}", name=f"h{k}")
                    for nb in range(NB):
                        sl = slice(512 * nb, 512 * (nb + 1))
                        if k == 2:
                            nc.vector.tensor_scalar(
                                out=ht[:, sl], in0=x_sb[k][:, sl],
                                scalar1=s_c[:], scalar2=b_c[:], op0=MUL,
                                op1=ADD)
                        else:
                            nc.scalar.activation(ht[:, sl], x_sb[k][:, sl],
                                                 IDENT, bias=b_c[:],
                                                 scale=s_c[:])
                    h_sb[k] = ht

                # ================= qk tiles =================
                qk_tiles = {}

                def gen_qk01():
                    # m = 0, 1 interleaved nb-major so QK(0) steps on the
                    # first t-half can start as early as possible
                    pqs = [psA.tile([128, T], F32, tag="big", name="pq")
                           for _ in range(2)]
                    for m in range(2):
                        qk_tiles[m] = singles.tile(
                            [128, T], BF16, tag=f"qk{m}{sfx}", name=f"qk{m}")
                    for nb in range(NB):
                        sl = slice(512 * nb, 512 * (nb + 1))
                        for m in range(2):
                            for k in range(NCHUNK):
                                nc.tensor.matmul(
                                    pqs[m][:, sl], wqk_ap(k, m),
                                    h_sb[k][:, sl], start=(k == 0),
                                    stop=(k == 3))
                        nc.scalar.activation(qk_tiles[0][:, sl], pqs[0][:, sl],
                                             IDENT, bias=bqk_sb[:, 0:1])
                        nc.vector.tensor_scalar(
                            out=qk_tiles[1][:, sl], in0=pqs[1][:, sl],
                            scalar1=bqk_sb[:, 1:2], scalar2=None, op0=ADD)

                gen_qk01()

                # ================= attention state =================
                ew_pairs = {}

                def _ew(h, j):
                    if (h, j) not in ew_pairs:
                        ew_pairs[(h, j)] = ewp.tile([128, 2, T], FP8, tag="ew",
                                                    name=f"ew{h}_{j}")
                    return ew_pairs[(h, j)]

                def emit_qk_step(h, st):
                    # logits for head h, s-tile st: 2 matmuls + 1 merged exp
                    p, slot = h // 2, h % 2
                    lo, hi = CH * slot, CH * (slot + 1)
                    ktile, qtile = qk_tiles[2 * p], qk_tiles[2 * p + 1]
                    j, pl = st // 2, st % 2
                    pw = psB.tile([128, T], F32, tag="pw", name="pw")
                    for nb in range(NB):
                        nc.tensor.matmul(
                            pw[:, 512 * nb:512 * (nb + 1)],
                            ktile[lo:hi, 128 * st:128 * (st + 1)],
                            qtile[lo:hi, 512 * nb:512 * (nb + 1)],
                            start=True, stop=True)
                    nc.scalar.activation(_ew(h, j)[:, pl, :], pw[:], EXP)

                def emit_qk_half_step(h, st, nb):
                    # one t-half of head h's logits (used to stretch head 7's
                    # exp stream over the last two windows)
                    p, slot = h // 2, h % 2
                    lo, hi = CH * slot, CH * (slot + 1)
                    ktile, qtile = qk_tiles[2 * p], qk_tiles[2 * p + 1]
                    j, pl = st // 2, st % 2
                    sl = slice(512 * nb, 512 * (nb + 1))
                    pw = psB.tile([128, 512], F32, tag="pw", name="pwh")
                    nc.tensor.matmul(
                        pw[:], ktile[lo:hi, 128 * st:128 * (st + 1)],
                        qtile[lo:hi, sl], start=True, stop=True)
                    nc.scalar.activation(_ew(h, j)[:, pl, sl], pw[:], EXP)

                # vt pair tiles (fp8, rider block cols 0:RID with col0 = ones)
                vt_sb = [singles.tile([128, 2, H, RID + CH], FP8,
                                      tag=f"vt{j}", name=f"vt{j}")
                         for j in range(NJ)]
                for j in range(NJ):
                    nc.vector.memset(vt_sb[j][:, :, :, 0:RID], 0.0)
                    nc.vector.memset(vt_sb[j][:, :, :, 0:1], 1.0)

                def emit_v_tile(st):
                    pv = psA.tile([128, C], F32, tag="big", name="pv")
                    for k in range(NCHUNK):
                        nc.tensor.matmul(pv[:],
                                         h_sb[k][:, 128 * st:128 * (st + 1)],
                                         wv_ap(k), start=(k == 0),
                                         stop=(k == 3))
                    nc.vector.tensor_copy(
                        vt_sb[st // 2][:, st % 2, :, RID:RID + CH],
                        pv[:].rearrange("p (h c) -> p h c", h=H))

                a_sb = [singles.tile([128, T], BF16, tag=f"a{p}",
                                     name=f"a{p}{sfx}") for p in range(NCHUNK)]
                acc_sb = [singles.tile([128, T], F32, tag=f"acc{m}",
                                       name=f"acc{m}{sfx}")
                          for m in range(NCHUNK)]

                # ---- prologue: v tiles + QK(0) steps (pv uses the big tag
                # ---- so the pw rotation stays a pure QK/exp double-buffer)
                for st in range(NT):
                    emit_qk_step(0, st)
                    emit_v_tile(st)

                # ================= duties =================
                def qk_spread_duty(m):
                    pq = psA.tile([128, T], F32, tag="big", name="pq")
                    qk = singles.tile([128, T], BF16, tag=f"qk{m}{sfx}",
                                      name=f"qk{m}")
                    qk_tiles[m] = qk
                    for nb in range(NB):
                        sl = slice(512 * nb, 512 * (nb + 1))
                        for k in range(NCHUNK):
                            nc.tensor.matmul(
                                pq[:, sl], wqk_ap(k, m), h_sb[k][:, sl],
                                start=(k == 0), stop=(k == 3))
                            yield
                        nc.vector.tensor_scalar(
                            out=qk[:, sl], in0=pq[:, sl],
                            scalar1=bqk_sb[:, m:m + 1], scalar2=None, op0=ADD)
                    yield

                def proj01_duty():
                    # acc[m] = (Wp0 @ a0 + Wp1 @ a1 + bproj) + x
                    for m in range(NCHUNK):
                        po = psA.tile([128, T], F32, tag="big", name="po")
                        for nb in range(NB):
                            sl = slice(512 * nb, 512 * (nb + 1))
                            nc.tensor.matmul(
                                po[:, sl], wproj_ap(0, m), a_sb[0][:, sl],
                                start=True, stop=False)
                            yield
                            nc.tensor.matmul(
                                po[:, sl], wproj_ap(1, m), a_sb[1][:, sl],
                                start=False, stop=True)
                            yield
                        nc.vector.scalar_tensor_tensor(
                            out=acc_sb[m][:], in0=po[:],
                            scalar=bproj_sb[:, m:m + 1], in1=x_sb[m][:],
                            op0=ADD, op1=ADD)
                        yield

                def proj2_duty(mlo, mhi):
                    # acc[m] += Wp2 @ a2
                    for m in range(mlo, mhi):
                        po = psA.tile([128, T], F32, tag="big", name="po")
                        for nb in range(NB):
                            sl = slice(512 * nb, 512 * (nb + 1))
                            nc.tensor.matmul(
                                po[:, sl], wproj_ap(2, m), a_sb[2][:, sl],
                                start=True, stop=True)
                            yield
                        nc.vector.tensor_add(acc_sb[m][:], po[:], acc_sb[m][:])
                        yield

                # ================= head loop =================
                last_rep = rep == n_reps - 1
                for h in range(H):
                    p, slot = h // 2, h % 2
                    lo, hi = CH * slot, CH * (slot + 1)
                    duties = []
                    if h == 0:
                        duties.append(qk_spread_duty(2))
                        duties.append(qk_spread_duty(3))
                    elif h == 1:
                        duties.append(qk_spread_duty(4))
                    elif h == 2:
                        duties.append(qk_spread_duty(5))
                    elif h == 3:
                        duties.append(qk_spread_duty(6))
                    elif h == 4:
                        duties.append(qk_spread_duty(7))
                    elif h == 5:
                        duties.append(proj01_duty())
                    elif h == 6:
                        duties.append(proj2_duty(0, 2))
                    pa = psA.tile([128, T], F32, tag="big", name="pa")

                    def tail_nb(nb, pa=pa):
                        # normalize head 7's nb half, then pair-3 proj + out
                        sl = slice(512 * nb, 512 * (nb + 1))
                        rcb = recp.tile([1, 512], F32, tag="rcb", name="rcb")
                        nc.vector.reciprocal_approx_fast(rcb[:], pa[0:1, sl])
                        rbb = rbp.tile([CH, 512], F32, tag="rbb", name="rbb")
                        nc.gpsimd.partition_broadcast(rbb[:], rcb[:])
                        nc.vector.tensor_mul(
                            a_sb[3][CH:128, sl], pa[RID:RID + CH, sl], rbb[:])
                        yield
                        for m in range(NCHUNK):
                            po = psA.tile([128, 512], F32, tag="big",
                                          name="pot")
                            nc.tensor.matmul(po[:], wproj_ap(3, m),
                                             a_sb[3][:, sl],
                                             start=True, stop=True)
                            nc.vector.tensor_add(acc_sb[m][:, sl], po[:],
                                                 acc_sb[m][:, sl])
                            if last_rep:
                                nc.sync.dma_start(
                                    out_d[128 * m:128 * (m + 1), sl],
                                    acc_sb[m][:, sl])
                            yield

                    if h == 7:
                        duties.append(proj2_duty(2, 4))
                        # nb0 logits were made during window 6: all nb0 AV now
                        for j in range(NJ):
                            nc.tensor.matmul(
                                pa[:, 0:512], vt_sb[j][:, :, 7, :],
                                ew_pairs[(7, j)][:, :, 0:512],
                                start=(j == 0), stop=(j == NJ - 1),
                                perf_mode=DR)
                        duties.append(tail_nb(0))
                    else:
                        # ew for this head is complete: AV burst at window
                        # start, normalization runs mid-window
                        for j in range(NJ):
                            ewt = ew_pairs.pop((h, j))
                            for nb in range(NB):
                                nc.tensor.matmul(
                                    pa[:, 512 * nb:512 * (nb + 1)],
                                    vt_sb[j][:, :, h, :],
                                    ewt[:, :, 512 * nb:512 * (nb + 1)],
                                    start=(j == 0), stop=(j == NJ - 1),
                                    perf_mode=DR)
                        rec = recp.tile([1, T], F32, tag="rec", name="rec")
                        nc.vector.reciprocal_approx_fast(rec[:], pa[0:1, :])
                        rb = rbp.tile([CH, T], F32, tag="rb", name="rb")
                        nc.gpsimd.partition_broadcast(rb[:], rec[:])
                        nc.vector.tensor_mul(a_sb[p][lo:hi, :],
                                             pa[RID:RID + CH, :], rb[:])
                    for st in range(NT):
                        if h < 6:
                            emit_qk_step(h + 1, st)
                        elif h == 6:
                            emit_qk_half_step(7, st, 0)
                        else:
                            emit_qk_half_step(7, st, 1)
                        advanced = 0
                        while duties and advanced < 3:
                            try:
                                next(duties[0])
                                advanced += 1
                            except StopIteration:
                                duties.pop(0)
                        if h == 7 and st % 2 == 1:
                            j = st // 2
                            ewt = ew_pairs.pop((7, j))
                            nc.tensor.matmul(
                                pa[:, 512:1024], vt_sb[j][:, :, 7, :],
                                ewt[:, :, 512:1024],
                                start=(j == 0), stop=(j == NJ - 1),
                                perf_mode=DR)
                    for g in duties:
                        for _ in g:
                            pass
                    if h == 7:
                        for _ in tail_nb(1):
                            pass

    nc.compile()
    return nc


def _get_program(n_reps=1):
    key = ("prog", n_reps)
    if key not in _CACHE:
        _CACHE[key] = _build_program(n_reps)
    return _CACHE[key]


def kernel(x, gn_w, gn_b, qkv_w, qkv_b, proj_w, proj_b, _n_reps=1):
    x = np.asarray(x, dtype=np.float32)
    hw = _host_weights(np.asarray(gn_w, np.float32), np.asarray(gn_b, np.float32),
                       np.asarray(qkv_w, np.float32), np.asarray(qkv_b, np.float32),
                       np.asarray(proj_w, np.float32), np.asarray(proj_b, np.float32))
    xr = np.ascontiguousarray(x.reshape(B, C, T))
    nc = _get_program(_n_reps)
    in_maps = [dict(hw, x=xr[b]) for b in range(B)]
    res = run_bass_kernel_spmd(nc, in_maps, core_ids=list(range(B)))
    out = np.stack([res.results[b]["out"] for b in range(B)])
    return out.reshape(B, C, HS, WS).astype(np.float32)


# revision 15
# speedup vs baseline: 1.0606x; 1.0144x over previous
"""Trainium2 Bass kernel for MultiHeadSelfAttention (GroupNorm + QKV + attention + proj + residual).

Problem shape (hardcoded): x [8, 512, 32, 32] fp32, 8 heads, 32 groups.
Sharding: data-parallel over batch B=8 across the 8 NeuronCores (one batch per core).

Per-core pipeline (T = 1024 positions, C = 512 channels, ch = 64 per head):
  1. GroupNorm(32) chunk-pipelined: groups never cross a 128-channel chunk;
     rsqrt(var+eps) via the quake bit-hack + 2 Newton steps on DVE so the
     Activation engine runs softmax exps only (no act-table swaps).
  2. qkv = qkv_w @ h with host-reordered bf16 weights:
       - q,k tiles [128, T]: m-tile 2p = [k_h(2p)|k_h(2p+1)], 2p+1 = [q...]
       - v produced transposed per s-tile, packed as fp8e4 pairs for DoubleRow
  3. Per head: logits via PE (bf16), ONE merged exp [128,1024] per s-tile on
     ACT writing fp8e4 straight to SBUF; attention @ V via fp8 DoubleRow
     matmuls (two s-planes per instruction; 64-col rider block carries the
     softmax denominator in partition 0); DVE reciprocal + gpsimd
     partition_broadcast + DVE mul to normalize.
  4. proj accumulated in PSUM per pair-group; v-bias folded into the proj
     bias on the host; bias+residual fused via scalar_tensor_tensor. Only
     the head-7 contraction half remains for the tail.

All input DMAs ride one ordered SP queue (x + wqk first) so the first
softmax exp lands as early as possible; the exp stream is the critical
resource and runs back-to-back for the rest of the kernel.
"""

import ml_dtypes
import numpy as np

import concourse.bass as bass
import concourse.bacc as bacc
import concourse.tile as tile
import concourse.mybir as mybir
from concourse import library_config
from concourse.bass_utils import run_bass_kernel_spmd

B, C, HS, WS = 8, 512, 32, 32
T = HS * WS            # 1024
H = 8                  # heads
CH = C // H            # 64
G = 32                 # groups
CPG = C // G           # 16 channels per group
EPS = 1e-5
NCHUNK = C // 128      # 4 channel chunks
NT = T // 128          # 8 sequence tiles
NB = T // 512          # 2 psum banks over T
NJ = NT // 2           # 4 s-tile pairs (DoubleRow planes)
RID = 64               # rider cols per head (col 0 = ones); out partitions 128
MAGIC = 0x5F3759DF     # quake rsqrt seed
F32 = mybir.dt.float32
F32R = mybir.dt.float32r
I32 = mybir.dt.int32
BF16 = mybir.dt.bfloat16
FP8 = mybir.dt.float8e4
EXP = mybir.ActivationFunctionType.Exp
IDENT = mybir.ActivationFunctionType.Identity
DR = mybir.MatmulPerfMode.DoubleRow
MUL = mybir.AluOpType.mult
ADD = mybir.AluOpType.add
SHR = mybir.AluOpType.logical_shift_right

_CACHE = {}
KORD = (1, 2, 0, 3)


def _orig_row(kind, h, i):
    off = {"q": 0, "k": CH, "v": 2 * CH}[kind]
    return 192 * h + off + i


def _host_weights(gn_w, gn_b, qkv_w, qkv_b, proj_w, proj_b):
    scale2 = 1.0 / np.sqrt(CH)  # ch**-0.25 on both q and k -> fold into k
    rows = np.zeros(2 * C, dtype=np.int64)
    colscale = np.ones(2 * C, dtype=np.float32)
    for p in range(H // 2):
        for slot in range(2):
            h = 2 * p + slot
            for i in range(CH):
                col_k = (2 * p) * 128 + slot * CH + i
                rows[col_k] = _orig_row("k", h, i)
                colscale[col_k] = scale2
                col_q = (2 * p + 1) * 128 + slot * CH + i
                rows[col_q] = _orig_row("q", h, i)
    wqk = (qkv_w[rows, :] * colscale[:, None]).T.copy()      # [512, 1024]
    # two DMA tiles: chunks (0,1) and (2,3) side by side
    wqk_t = np.ascontiguousarray(
        wqk.reshape(2, 2, 128, 2 * C).transpose(0, 2, 1, 3).reshape(
            2, 128, 4 * C)).astype(ml_dtypes.bfloat16)
    bqk = (qkv_b[rows] * colscale).reshape(8, 128).T.copy()  # [128, 8]

    vrows = np.array([_orig_row("v", h, i) for h in range(H) for i in range(CH)])
    wv = qkv_w[vrows, :].T.copy()                            # [512, 512] (c, c_v)
    wv_t = np.ascontiguousarray(
        wv.reshape(NCHUNK, 128, C).transpose(1, 0, 2).reshape(
            128, NCHUNK * C)).astype(ml_dtypes.bfloat16)     # [128, 2048]

    bv = qkv_b[vrows]
    bproj_full = proj_b + proj_w @ bv                        # [512]
    wproj = proj_w.T.copy()                                  # [512(c), 512(o)]
    wproj_t = np.ascontiguousarray(
        wproj.reshape(NCHUNK, 128, C).transpose(1, 0, 2).reshape(
            128, NCHUNK * C)).astype(ml_dtypes.bfloat16)

    # consolidated f32 consts [128, 24]: g8 | gnw | gnb | bqk | bproj
    g8 = np.zeros((128, 8), dtype=np.float32)
    gt8 = np.zeros((8, 128), dtype=np.float32)
    for u in range(128):
        g8[u, u // CPG] = 1.0 / CPG
        gt8[u // CPG, u] = 1.0
    cst = np.concatenate([
        g8,
        gn_w.reshape(NCHUNK, 128).T,
        gn_b.reshape(NCHUNK, 128).T,
        bqk,
        bproj_full.reshape(NCHUNK, 128).T,
    ], axis=1).astype(np.float32)                            # [128, 28]
    return {"cst": cst, "gt8": gt8, "wqk": wqk_t, "wv": wv_t,
            "wproj": wproj_t}


def _build_program(n_reps=1, ew_bufs=12):
    nc = bacc.Bacc("TRN2", target_bir_lowering=False, debug=False, num_devices=8)
    dt_in = [
        ("x", [C, T], F32), ("cst", [128, 28], F32R), ("gt8", [8, 128], F32R),
        ("wqk", [2, 128, 4 * C], BF16), ("wv", [128, NCHUNK * C], BF16),
        ("wproj", [128, NCHUNK * C], BF16),
    ]
    d = {name: nc.dram_tensor(name, shape, dt, kind="ExternalInput").ap()
         for name, shape, dt in dt_in}
    out_d = nc.dram_tensor("out", [C, T], F32, kind="ExternalOutput").ap()

    with tile.TileContext(nc) as tc:
        with (
            tc.tile_pool(name="singles", bufs=1) as singles,
            tc.tile_pool(name="small", bufs=16) as small,
            tc.tile_pool(name="ewp", bufs=ew_bufs) as ewp,
            tc.tile_pool(name="recp", bufs=2) as recp,
            tc.tile_pool(name="rbp", bufs=2) as rbp,
            tc.tile_pool(name="psA", bufs=2, space="PSUM") as psA,
            tc.tile_pool(name="psB", bufs=2, space="PSUM") as psB,
        ):
            nc.gpsimd.load_library(library_config.attn)

            # ---- one ordered DMA stream on the SP queue: consts, then x
            # ---- halves interleaved with wqk, then wv/wproj ----
            cst = singles.tile([128, 28], F32R, tag="cst", name="cst")
            nc.sync.dma_start(cst[:], d["cst"][:])
            gt8_sb = singles.tile([8, 128], F32R, tag="gt8", name="gt8")
            nc.sync.dma_start(gt8_sb[:], d["gt8"][:])
            g8_sb = cst[:, 0:8]
            gnw_sb = cst[:, 8:12].bitcast(F32)
            gnb_sb = cst[:, 12:16].bitcast(F32)
            bqk_sb = cst[:, 16:24].bitcast(F32)
            bproj_sb = cst[:, 24:28].bitcast(F32)

            x_sb = [singles.tile([128, T], F32, tag=f"x{k}", name=f"x{k}")
                    for k in range(NCHUNK)]
            wqk_sb = [singles.tile([128, 4 * C], BF16, tag=f"wqk{g}",
                                   name=f"wqk{g}") for g in range(2)]
            for k in range(NCHUNK):
                for nb in range(NB):
                    sl = slice(512 * nb, 512 * (nb + 1))
                    nc.sync.dma_start(x_sb[k][:, sl],
                                      d["x"][128 * k:128 * (k + 1), sl])
            for g in range(2):
                nc.sync.dma_start(wqk_sb[g][:], d["wqk"][g])
            wv_sb = singles.tile([128, NCHUNK * C], BF16, tag="wv", name="wv")
            nc.sync.dma_start(wv_sb[:], d["wv"][:])
            wproj_sb = singles.tile([128, NCHUNK * C], BF16, tag="wproj",
                                    name="wproj")
            nc.sync.dma_start(wproj_sb[:], d["wproj"][:])

            def wqk_ap(k, m):
                # chunk k, m-tile column block [128, 128]
                return wqk_sb[k // 2][:, 1024 * (k % 2) + 128 * m:
                                      1024 * (k % 2) + 128 * (m + 1)]

            def wv_ap(k):
                return wv_sb[:, 512 * k:512 * (k + 1)]

            def wproj_ap(p, m, clo=0, chi=128):
                return wproj_sb[clo:chi, 512 * p + 128 * m:512 * p + 128 * (m + 1)]

            magic_t = singles.tile([8, 1], I32, tag="magic", name="magic")
            nc.vector.memset(magic_t[:], MAGIC)
            # prime the exp/identity activation table while ACT is idle
            prim = singles.tile([1, 1], F32, tag="prim", name="prim")
            nc.vector.memset(prim[:], 0.0)
            nc.scalar.activation(prim[:], prim[:], IDENT)

            for rep in range(n_reps):
                sfx = f"r{rep}"
                # ================= GroupNorm (per chunk) =================
                h_sb = [None] * NCHUNK
                stats_t = {}
                # phase 1: per-channel stats -- chunks 2,3 on DVE first (the
                # late-arriving x tiles), chunks 0,1 via ACT accumulators
                for k in (1, 2, 3):
                    stats = small.tile([128, 2], F32R, tag="small", name="stats")
                    st6 = small.tile([128, 2, 6], F32, tag="small", name="st6")
                    nc.vector.bn_stats(st6[:, 0, :], x_sb[k][:, 0:512])
                    nc.vector.bn_stats(st6[:, 1, :], x_sb[k][:, 512:1024])
                    mv = small.tile([128, 2], F32, tag="small", name="mv")
                    nc.vector.bn_aggr(mv[:], st6[:])
                    m2 = small.tile([128, 1], F32, tag="small", name="m2")
                    nc.vector.tensor_mul(m2[:], mv[:, 0:1], mv[:, 0:1])
                    nc.vector.tensor_copy(stats[:, 0:1], mv[:, 0:1])
                    nc.vector.tensor_add(stats[:, 1:2], mv[:, 1:2], m2[:])
                    stats_t[k] = stats
                for k in (0,):
                    stats = small.tile([128, 2], F32R, tag="small", name="stats")
                    scr = small.tile([128, T], BF16, tag="gnscr", bufs=2,
                                     name="scr")
                    asm = small.tile([128, 1], F32, tag="small", name="asm")
                    nc.scalar.activation(scr[:], x_sb[k][:], IDENT,
                                         accum_out=asm[:])
                    asq = small.tile([128, 1], F32, tag="small", name="asq")
                    nc.scalar.activation(scr[:], x_sb[k][:],
                                         mybir.ActivationFunctionType.Square,
                                         accum_out=asq[:])
                    nc.vector.tensor_scalar(out=stats[:, 0:1], in0=asm[:],
                                            scalar1=1.0 / T, scalar2=None,
                                            op0=MUL)
                    nc.vector.tensor_scalar(out=stats[:, 1:2], in0=asq[:],
                                            scalar1=1.0 / T, scalar2=None,
                                            op0=MUL)
                    stats_t[k] = stats
                # phase 2: group combine + affine, in stats-arrival order
                for k in (1, 2, 0, 3):
                    stats = stats_t[k]
                    psg = psA.tile([8, 2], F32, tag="big", name="psg")
                    nc.tensor.matmul(psg[:], g8_sb, stats[:],
                                     start=True, stop=True)
                    gsb = small.tile([8, 2], F32, tag="small", name="gsb")
                    nc.vector.tensor_copy(gsb[:], psg[:])
                    mu2 = small.tile([8, 1], F32, tag="small", name="mu2")
                    nc.vector.tensor_mul(mu2[:], gsb[:, 0:1], gsb[:, 0:1])
                    av = small.tile([8, 1], F32, tag="small", name="av")
                    nc.vector.tensor_sub(av[:], gsb[:, 1:2], mu2[:])
                    nc.vector.tensor_scalar(out=av[:], in0=av[:], scalar1=EPS,
                                            scalar2=None, op0=ADD)
                    yi = small.tile([8, 1], I32, tag="small", name="yi")
                    nc.vector.tensor_scalar(out=yi[:], in0=av[:].bitcast(I32),
                                            scalar1=1, scalar2=None, op0=SHR)
                    nc.vector.tensor_sub(yi[:], magic_t[:], yi[:])
                    y = yi[:].bitcast(F32)
                    ah = small.tile([8, 1], F32, tag="small", name="ah")
                    nc.vector.tensor_scalar(out=ah[:], in0=av[:], scalar1=0.5,
                                            scalar2=None, op0=MUL)
                    t2 = small.tile([8, 1], F32, tag="small", name="t2")
                    nc.vector.tensor_mul(t2[:], y, y)
                    nc.vector.tensor_mul(t2[:], t2[:], ah[:])
                    nc.vector.tensor_scalar(out=t2[:], in0=t2[:],
                                            scalar1=-1.0, scalar2=1.5,
                                            op0=MUL, op1=ADD)
                    nc.vector.tensor_mul(y, y, t2[:])
                    grp = small.tile([8, 2], F32R, tag="small", name="grp")
                    nc.vector.tensor_copy(grp[:, 0:1], gsb[:, 0:1])
                    nc.vector.tensor_copy(grp[:, 1:2], y)
                    psc = psA.tile([128, 2], F32, tag="big", name="psc")
                    nc.tensor.matmul(psc[:], gt8_sb[:], grp[:],
                                     start=True, stop=True)
                    s_c = small.tile([128, 1], F32, tag="small", name="s_c")
                    nc.vector.tensor_mul(s_c[:], psc[:, 1:2], gnw_sb[:, k:k + 1])
                    t1 = small.tile([128, 1], F32, tag="small", name="t1")
                    nc.vector.tensor_mul(t1[:], psc[:, 0:1], s_c[:])
                    b_c = small.tile([128, 1], F32, tag="small", name="b_c")
                    nc.vector.tensor_sub(b_c[:], gnb_sb[:, k:k + 1], t1[:])
                    ht = singles.tile([128, T], BF16, tag=f"h{k}", name=f"h{k}")
                    for nb in range(NB):
                        sl = slice(512 * nb, 512 * (nb + 1))
                        if k == 3:
                            nc.vector.tensor_scalar(
                                out=ht[:, sl], in0=x_sb[k][:, sl],
                                scalar1=s_c[:], scalar2=b_c[:], op0=MUL,
                                op1=ADD)
                        else:
                            nc.scalar.activation(ht[:, sl], x_sb[k][:, sl],
                                                 IDENT, bias=b_c[:],
                                                 scale=s_c[:])
                    h_sb[k] = ht

                # ================= qk tiles =================
                qk_tiles = {}

                def gen_qk01():
                    # m = 0, 1 interleaved nb-major so QK(0) steps on the
                    # first t-half can start as early as possible
                    pqs = [psA.tile([128, T], F32, tag="big", name="pq")
                           for _ in range(2)]
                    for m in range(2):
                        qk_tiles[m] = singles.tile(
                            [128, T], BF16, tag=f"qk{m}{sfx}", name=f"qk{m}")
                    for nb in range(NB):
                        sl = slice(512 * nb, 512 * (nb + 1))
                        for m in range(2):
                            for i, k in enumerate(KORD):
                                nc.tensor.matmul(
                                    pqs[m][:, sl], wqk_ap(k, m),
                                    h_sb[k][:, sl], start=(i == 0),
                                    stop=(i == 3))
                        nc.scalar.activation(qk_tiles[0][:, sl], pqs[0][:, sl],
                                             IDENT, bias=bqk_sb[:, 0:1])
                        nc.vector.tensor_scalar(
                            out=qk_tiles[1][:, sl], in0=pqs[1][:, sl],
                            scalar1=bqk_sb[:, 1:2], scalar2=None, op0=ADD)

                gen_qk01()

                # ================= attention state =================
                ew_pairs = {}

                def _ew(h, j):
                    if (h, j) not in ew_pairs:
                        ew_pairs[(h, j)] = ewp.tile([128, 2, T], FP8, tag="ew",
                                                    name=f"ew{h}_{j}")
                    return ew_pairs[(h, j)]

                def emit_qk_step(h, st):
                    # logits for head h, s-tile st: 2 matmuls + 1 merged exp
                    p, slot = h // 2, h % 2
                    lo, hi = CH * slot, CH * (slot + 1)
                    ktile, qtile = qk_tiles[2 * p], qk_tiles[2 * p + 1]
                    j, pl = st // 2, st % 2
                    pw = psB.tile([128, T], F32, tag="pw", name="pw")
                    for nb in range(NB):
                        nc.tensor.matmul(
                            pw[:, 512 * nb:512 * (nb + 1)],
                            ktile[lo:hi, 128 * st:128 * (st + 1)],
                            qtile[lo:hi, 512 * nb:512 * (nb + 1)],
                            start=True, stop=True)
                    nc.scalar.activation(_ew(h, j)[:, pl, :], pw[:], EXP)

                def emit_qk_half_step(h, st, nb):
                    # one t-half of head h's logits (used to stretch head 7's
                    # exp stream over the last two windows)
                    p, slot = h // 2, h % 2
                    lo, hi = CH * slot, CH * (slot + 1)
                    ktile, qtile = qk_tiles[2 * p], qk_tiles[2 * p + 1]
                    j, pl = st // 2, st % 2
                    sl = slice(512 * nb, 512 * (nb + 1))
                    pw = psB.tile([128, 512], F32, tag="pw", name="pwh")
                    nc.tensor.matmul(
                        pw[:], ktile[lo:hi, 128 * st:128 * (st + 1)],
                        qtile[lo:hi, sl], start=True, stop=True)
                    nc.scalar.activation(_ew(h, j)[:, pl, sl], pw[:], EXP)

                # vt pair tiles (fp8, rider block cols 0:RID with col0 = ones)
                vt_sb = [singles.tile([128, 2, H, RID + CH], FP8,
                                      tag=f"vt{j}", name=f"vt{j}")
                         for j in range(NJ)]
                for j in range(NJ):
                    nc.vector.memset(vt_sb[j][:, :, :, 0:RID], 0.0)
                    nc.vector.memset(vt_sb[j][:, :, :, 0:1], 1.0)

                def emit_v_tile(st):
                    pv = psA.tile([128, C], F32, tag="big", name="pv")
                    for i, k in enumerate(KORD):
                        nc.tensor.matmul(pv[:],
                                         h_sb[k][:, 128 * st:128 * (st + 1)],
                                         wv_ap(k), start=(i == 0),
                                         stop=(i == 3))
                    nc.vector.tensor_copy(
                        vt_sb[st // 2][:, st % 2, :, RID:RID + CH],
                        pv[:].rearrange("p (h c) -> p h c", h=H))

                a_sb = [singles.tile([128, T], BF16, tag=f"a{p}",
                                     name=f"a{p}{sfx}") for p in range(NCHUNK)]
                acc_sb = [singles.tile([128, T], F32, tag=f"acc{m}",
                                       name=f"acc{m}{sfx}")
                          for m in range(NCHUNK)]

                # ---- prologue: v tiles + QK(0) steps (pv uses the big tag
                # ---- so the pw rotation stays a pure QK/exp double-buffer)
                for st in range(NT):
                    if st < 2:
                        emit_qk_half_step(0, st, 0)
                        emit_qk_half_step(0, st, 1)
                    else:
                        emit_qk_step(0, st)
                    emit_v_tile(st)

                # ================= duties =================
                def qk_spread_duty(m):
                    pq = psA.tile([128, T], F32, tag="big", name="pq")
                    qk = singles.tile([128, T], BF16, tag=f"qk{m}{sfx}",
                                      name=f"qk{m}")
                    qk_tiles[m] = qk
                    for nb in range(NB):
                        sl = slice(512 * nb, 512 * (nb + 1))
                        for i, k in enumerate(KORD):
                            nc.tensor.matmul(
                                pq[:, sl], wqk_ap(k, m), h_sb[k][:, sl],
                                start=(i == 0), stop=(i == 3))
                            yield
                        nc.vector.tensor_scalar(
                            out=qk[:, sl], in0=pq[:, sl],
                            scalar1=bqk_sb[:, m:m + 1], scalar2=None, op0=ADD)
                    yield

                def proj01_duty():
                    # acc[m] = (Wp0 @ a0 + Wp1 @ a1 + bproj) + x
                    for m in range(NCHUNK):
                        po = psA.tile([128, T], F32, tag="big", name="po")
                        for nb in range(NB):
                            sl = slice(512 * nb, 512 * (nb + 1))
                            nc.tensor.matmul(
                                po[:, sl], wproj_ap(0, m), a_sb[0][:, sl],
                                start=True, stop=False)
                            yield
                            nc.tensor.matmul(
                                po[:, sl], wproj_ap(1, m), a_sb[1][:, sl],
                                start=False, stop=True)
                            yield
                        nc.vector.scalar_tensor_tensor(
                            out=acc_sb[m][:], in0=po[:],
                            scalar=bproj_sb[:, m:m + 1], in1=x_sb[m][:],
                            op0=ADD, op1=ADD)
                        yield

                def proj2_duty(mlo, mhi):
                    # acc[m] += Wp2 @ a2
                    for m in range(mlo, mhi):
                        po = psA.tile([128, T], F32, tag="big", name="po")
                        for nb in range(NB):
                            sl = slice(512 * nb, 512 * (nb + 1))
                            nc.tensor.matmul(
                                po[:, sl], wproj_ap(2, m), a_sb[2][:, sl],
                                start=True, stop=True)
                            yield
                        nc.vector.tensor_add(acc_sb[m][:], po[:], acc_sb[m][:])
                        yield

                # ================= head loop =================
                last_rep = rep == n_reps - 1
                for h in range(H):
                    p, slot = h // 2, h % 2
                    lo, hi = CH * slot, CH * (slot + 1)
                    duties = []
                    if h == 0:
                        duties.append(qk_spread_duty(2))
                        duties.append(qk_spread_duty(3))
                    elif h == 1:
                        duties.append(qk_spread_duty(4))
                    elif h == 2:
                        duties.append(qk_spread_duty(5))
                    elif h == 3:
                        duties.append(qk_spread_duty(6))
                    elif h == 4:
                        duties.append(qk_spread_duty(7))
                    elif h == 5:
                        duties.append(proj01_duty())
                    elif h == 6:
                        duties.append(proj2_duty(0, 4))
                    pa = psA.tile([128, T], F32, tag="big", name="pa")
                    # feed ACT before the AV burst: first two qk steps
                    if h < 7:
                        emit_qk_step(h + 1, 0)
                        emit_qk_step(h + 1, 1)
                    # ew for this head is complete: AV burst, then normalize
                    for j in range(NJ):
                        ewt = ew_pairs.pop((h, j))
                        for nb in range(NB):
                            nc.tensor.matmul(
                                pa[:, 512 * nb:512 * (nb + 1)],
                                vt_sb[j][:, :, h, :],
                                ewt[:, :, 512 * nb:512 * (nb + 1)],
                                start=(j == 0), stop=(j == NJ - 1),
                                perf_mode=DR)
                    if h == H - 1:
                        # straight-line tail: normalize + pair-3 proj + out
                        for nb in range(NB):
                            sl = slice(512 * nb, 512 * (nb + 1))
                            rcb = recp.tile([1, 512], F32, tag="rcb",
                                            name="rcb")
                            nc.vector.reciprocal_approx_fast(rcb[:],
                                                             pa[0:1, sl])
                            rbb = rbp.tile([CH, 512], F32, tag="rbb",
                                           name="rbb")
                            nc.gpsimd.partition_broadcast(rbb[:], rcb[:])
                            nc.vector.tensor_mul(
                                a_sb[3][CH:128, sl], pa[RID:RID + CH, sl],
                                rbb[:])
                        for nb in range(NB):
                            sl = slice(512 * nb, 512 * (nb + 1))
                            for m in range(NCHUNK):
                                po = psA.tile([128, 512], F32, tag="big",
                                              name="pot")
                                nc.tensor.matmul(po[:], wproj_ap(3, m),
                                                 a_sb[3][:, sl],
                                                 start=True, stop=True)
                                nc.vector.tensor_add(acc_sb[m][:, sl], po[:],
                                                     acc_sb[m][:, sl])
                                if last_rep:
                                    nc.sync.dma_start(
                                        out_d[128 * m:128 * (m + 1), sl],
                                        acc_sb[m][:, sl])
                        continue
                    rec = recp.tile([1, T], F32, tag="rec", name="rec")
                    nc.vector.reciprocal_approx_fast(rec[:], pa[0:1, :])
                    rb = rbp.tile([CH, T], F32, tag="rb", name="rb")
                    nc.gpsimd.partition_broadcast(rb[:], rec[:])
                    nc.vector.tensor_mul(a_sb[p][lo:hi, :],
                                         pa[RID:RID + CH, :], rb[:])
                    for st in range(2, NT):
                        emit_qk_step(h + 1, st)
                        advanced = 0
                        while duties and advanced < 4:
                            try:
                                next(duties[0])
                                advanced += 1
                            except StopIteration:
                                duties.pop(0)
                    for g in duties:
                        for _ in g:
                            pass

    nc.compile()
    return nc


def _get_program(n_reps=1):
    key = ("prog", n_reps)
    if key not in _CACHE:
        _CACHE[key] = _build_program(n_reps)
    return _CACHE[key]


def kernel(x, gn_w, gn_b, qkv_w, qkv_b, proj_w, proj_b, _n_reps=1):
    x = np.asarray(x, dtype=np.float32)
    hw = _host_weights(np.asarray(gn_w, np.float32), np.asarray(gn_b, np.float32),
                       np.asarray(qkv_w, np.float32), np.asarray(qkv_b, np.float32),
                       np.asarray(proj_w, np.float32), np.asarray(proj_b, np.float32))
    xr = np.ascontiguousarray(x.reshape(B, C, T))
    nc = _get_program(_n_reps)
    in_maps = [dict(hw, x=xr[b]) for b in range(B)]
    res = run_bass_kernel_spmd(nc, in_maps, core_ids=list(range(B)))
    out = np.stack([res.results[b]["out"] for b in range(B)])
    return out.reshape(B, C, HS, WS).astype(np.float32)


# revision 16
# speedup vs baseline: 1.0835x; 1.0216x over previous
"""Trainium2 Bass kernel for MultiHeadSelfAttention (GroupNorm + QKV + attention + proj + residual).

Problem shape (hardcoded): x [8, 512, 32, 32] fp32, 8 heads, 32 groups.
Sharding: data-parallel over batch B=8 across the 8 NeuronCores (one batch per core).

Per-core pipeline (T = 1024 positions, C = 512 channels, ch = 64 per head):
  1. GroupNorm(32) chunk-pipelined: groups never cross a 128-channel chunk;
     rsqrt(var+eps) via the quake bit-hack + 2 Newton steps on DVE so the
     Activation engine runs softmax exps only (no act-table swaps).
  2. qkv = qkv_w @ h with host-reordered bf16 weights:
       - q,k tiles [128, T]: m-tile 2p = [k_h(2p)|k_h(2p+1)], 2p+1 = [q...]
       - v produced transposed per s-tile, packed as fp8e4 pairs for DoubleRow
  3. Per head: logits via PE (bf16), ONE merged exp [128,1024] per s-tile on
     ACT writing fp8e4 straight to SBUF; attention @ V via fp8 DoubleRow
     matmuls (two s-planes per instruction; 64-col rider block carries the
     softmax denominator in partition 0); DVE reciprocal + gpsimd
     partition_broadcast + DVE mul to normalize.
  4. proj accumulated in PSUM per pair-group; v-bias folded into the proj
     bias on the host; bias+residual fused via scalar_tensor_tensor. Only
     the head-7 contraction half remains for the tail.

All input DMAs ride one ordered SP queue (x + wqk first) so the first
softmax exp lands as early as possible; the exp stream is the critical
resource and runs back-to-back for the rest of the kernel.
"""

import ml_dtypes
import numpy as np

import concourse.bass as bass
import concourse.bacc as bacc
import concourse.tile as tile
import concourse.mybir as mybir
from concourse import library_config
from concourse.bass_utils import run_bass_kernel_spmd

B, C, HS, WS = 8, 512, 32, 32
T = HS * WS            # 1024
H = 8                  # heads
CH = C // H            # 64
G = 32                 # groups
CPG = C // G           # 16 channels per group
EPS = 1e-5
NCHUNK = C // 128      # 4 channel chunks
NT = T // 128          # 8 sequence tiles
NB = T // 512          # 2 psum banks over T
NJ = NT // 2           # 4 s-tile pairs (DoubleRow planes)
RID = 64               # rider cols per head (col 0 = ones); out partitions 128
MAGIC = 0x5F3759DF     # quake rsqrt seed
F32 = mybir.dt.float32
F32R = mybir.dt.float32r
I32 = mybir.dt.int32
BF16 = mybir.dt.bfloat16
FP8 = mybir.dt.float8e4
EXP = mybir.ActivationFunctionType.Exp
IDENT = mybir.ActivationFunctionType.Identity
DR = mybir.MatmulPerfMode.DoubleRow
MUL = mybir.AluOpType.mult
ADD = mybir.AluOpType.add
SHR = mybir.AluOpType.logical_shift_right

_CACHE = {}
KORD = (0, 2, 1, 3)


def _orig_row(kind, h, i):
    off = {"q": 0, "k": CH, "v": 2 * CH}[kind]
    return 192 * h + off + i


def _host_weights(gn_w, gn_b, qkv_w, qkv_b, proj_w, proj_b):
    scale2 = 1.0 / np.sqrt(CH)  # ch**-0.25 on both q and k -> fold into k
    rows = np.zeros(2 * C, dtype=np.int64)
    colscale = np.ones(2 * C, dtype=np.float32)
    for p in range(H // 2):
        for slot in range(2):
            h = 2 * p + slot
            for i in range(CH):
                col_k = (2 * p) * 128 + slot * CH + i
                rows[col_k] = _orig_row("k", h, i)
                colscale[col_k] = scale2
                col_q = (2 * p + 1) * 128 + slot * CH + i
                rows[col_q] = _orig_row("q", h, i)
    wqk = (qkv_w[rows, :] * colscale[:, None]).T.copy()      # [512, 1024]
    # two DMA tiles: chunks (0,1) and (2,3) side by side
    wqk_t = np.ascontiguousarray(
        wqk.reshape(2, 2, 128, 2 * C).transpose(0, 2, 1, 3).reshape(
            2, 128, 4 * C)).astype(ml_dtypes.bfloat16)
    bqk = (qkv_b[rows] * colscale).reshape(8, 128).T.copy()  # [128, 8]

    vrows = np.array([_orig_row("v", h, i) for h in range(H) for i in range(CH)])
    wv = qkv_w[vrows, :].T.copy()                            # [512, 512] (c, c_v)
    wv_t = np.ascontiguousarray(
        wv.reshape(NCHUNK, 128, C).transpose(1, 0, 2).reshape(
            128, NCHUNK * C)).astype(ml_dtypes.bfloat16)     # [128, 2048]

    bv = qkv_b[vrows]
    bproj_full = proj_b + proj_w @ bv                        # [512]
    wproj = proj_w.T.copy()                                  # [512(c), 512(o)]
    wproj_t = np.ascontiguousarray(
        wproj.reshape(NCHUNK, 128, C).transpose(1, 0, 2).reshape(
            128, NCHUNK * C)).astype(ml_dtypes.bfloat16)

    # consolidated f32 consts [128, 24]: g8 | gnw | gnb | bqk | bproj
    g8 = np.zeros((128, 8), dtype=np.float32)
    gt8 = np.zeros((8, 128), dtype=np.float32)
    for u in range(128):
        g8[u, u // CPG] = 1.0 / CPG
        gt8[u // CPG, u] = 1.0
    cst = np.concatenate([
        g8,
        gn_w.reshape(NCHUNK, 128).T,
        gn_b.reshape(NCHUNK, 128).T,
        bqk,
        bproj_full.reshape(NCHUNK, 128).T,
    ], axis=1).astype(np.float32)                            # [128, 28]
    return {"cst": cst, "gt8": gt8, "wqk": wqk_t, "wv": wv_t,
            "wproj": wproj_t}


def _build_program(n_reps=1, ew_bufs=12):
    nc = bacc.Bacc("TRN2", target_bir_lowering=False, debug=False, num_devices=8)
    dt_in = [
        ("x", [C, T], F32), ("cst", [128, 28], F32R), ("gt8", [8, 128], F32R),
        ("wqk", [2, 128, 4 * C], BF16), ("wv", [128, NCHUNK * C], BF16),
        ("wproj", [128, NCHUNK * C], BF16),
    ]
    d = {name: nc.dram_tensor(name, shape, dt, kind="ExternalInput").ap()
         for name, shape, dt in dt_in}
    out_d = nc.dram_tensor("out", [C, T], F32, kind="ExternalOutput").ap()

    with tile.TileContext(nc) as tc:
        with (
            tc.tile_pool(name="singles", bufs=1) as singles,
            tc.tile_pool(name="small", bufs=16) as small,
            tc.tile_pool(name="ewp", bufs=ew_bufs) as ewp,
            tc.tile_pool(name="recp", bufs=2) as recp,
            tc.tile_pool(name="psA", bufs=2, space="PSUM") as psA,
            tc.tile_pool(name="psB", bufs=2, space="PSUM") as psB,
        ):
            # ---- one ordered DMA stream on the SP queue: consts, then x
            # ---- halves interleaved with wqk, then wv/wproj ----
            cst = singles.tile([128, 28], F32R, tag="cst", name="cst")
            nc.sync.dma_start(cst[:], d["cst"][:])
            gt8_sb = singles.tile([8, 128], F32R, tag="gt8", name="gt8")
            nc.sync.dma_start(gt8_sb[:], d["gt8"][:])
            g8_sb = cst[:, 0:8]
            gnw_sb = cst[:, 8:12].bitcast(F32)
            gnb_sb = cst[:, 12:16].bitcast(F32)
            bqk_sb = cst[:, 16:24].bitcast(F32)
            bproj_sb = cst[:, 24:28].bitcast(F32)

            x_sb = [singles.tile([128, T], F32, tag=f"x{k}", name=f"x{k}")
                    for k in range(NCHUNK)]
            wqk_sb = [singles.tile([128, 4 * C], BF16, tag=f"wqk{g}",
                                   name=f"wqk{g}") for g in range(2)]
            for k in range(NCHUNK):
                for nb in range(NB):
                    sl = slice(512 * nb, 512 * (nb + 1))
                    nc.sync.dma_start(x_sb[k][:, sl],
                                      d["x"][128 * k:128 * (k + 1), sl])
            for g in range(2):
                nc.sync.dma_start(wqk_sb[g][:], d["wqk"][g])
            wv_sb = singles.tile([128, NCHUNK * C], BF16, tag="wv", name="wv")
            nc.sync.dma_start(wv_sb[:], d["wv"][:])
            wproj_sb = singles.tile([128, NCHUNK * C], BF16, tag="wproj",
                                    name="wproj")
            nc.sync.dma_start(wproj_sb[:], d["wproj"][:])

            def wqk_ap(k, m):
                # chunk k, m-tile column block [128, 128]
                return wqk_sb[k // 2][:, 1024 * (k % 2) + 128 * m:
                                      1024 * (k % 2) + 128 * (m + 1)]

            def wv_ap(k):
                return wv_sb[:, 512 * k:512 * (k + 1)]

            def wproj_ap(p, m, clo=0, chi=128):
                return wproj_sb[clo:chi, 512 * p + 128 * m:512 * p + 128 * (m + 1)]

            magic_t = singles.tile([8, 1], I32, tag="magic", name="magic")
            nc.vector.memset(magic_t[:], MAGIC)
            # prime the exp/identity activation table while ACT is idle
            prim = singles.tile([1, 1], F32, tag="prim", name="prim")
            nc.vector.memset(prim[:], 0.0)
            nc.scalar.activation(prim[:], prim[:], IDENT)

            for rep in range(n_reps):
                sfx = f"r{rep}"
                # ================= GroupNorm (per chunk) =================
                h_sb = [None] * NCHUNK
                stats_t = {}
                # phase 1: per-channel stats -- chunks 2,3 on DVE first (the
                # late-arriving x tiles), chunks 0,1 via ACT accumulators
                for k in (2, 3):
                    stats = small.tile([128, 2], F32R, tag="small", name="stats")
                    st6 = small.tile([128, 2, 6], F32, tag="small", name="st6")
                    nc.vector.bn_stats(st6[:, 0, :], x_sb[k][:, 0:512])
                    nc.vector.bn_stats(st6[:, 1, :], x_sb[k][:, 512:1024])
                    mv = small.tile([128, 2], F32, tag="small", name="mv")
                    nc.vector.bn_aggr(mv[:], st6[:])
                    m2 = small.tile([128, 1], F32, tag="small", name="m2")
                    nc.vector.tensor_mul(m2[:], mv[:, 0:1], mv[:, 0:1])
                    nc.vector.tensor_copy(stats[:, 0:1], mv[:, 0:1])
                    nc.vector.tensor_add(stats[:, 1:2], mv[:, 1:2], m2[:])
                    stats_t[k] = stats
                for k in (0, 1):
                    stats = small.tile([128, 2], F32R, tag="small", name="stats")
                    scr = small.tile([128, T], BF16, tag="gnscr", bufs=2,
                                     name="scr")
                    asm = small.tile([128, 1], F32, tag="small", name="asm")
                    nc.scalar.activation(scr[:], x_sb[k][:], IDENT,
                                         accum_out=asm[:])
                    asq = small.tile([128, 1], F32, tag="small", name="asq")
                    nc.scalar.activation(scr[:], x_sb[k][:],
                                         mybir.ActivationFunctionType.Square,
                                         accum_out=asq[:])
                    nc.vector.tensor_scalar(out=stats[:, 0:1], in0=asm[:],
                                            scalar1=1.0 / T, scalar2=None,
                                            op0=MUL)
                    nc.vector.tensor_scalar(out=stats[:, 1:2], in0=asq[:],
                                            scalar1=1.0 / T, scalar2=None,
                                            op0=MUL)
                    stats_t[k] = stats
                # phase 2: group combine + affine, in stats-arrival order
                for k in (0, 2, 1, 3):
                    stats = stats_t[k]
                    psg = psA.tile([8, 2], F32, tag="big", name="psg")
                    nc.tensor.matmul(psg[:], g8_sb, stats[:],
                                     start=True, stop=True)
                    gsb = small.tile([8, 2], F32, tag="small", name="gsb")
                    nc.vector.tensor_copy(gsb[:], psg[:])
                    mu2 = small.tile([8, 1], F32, tag="small", name="mu2")
                    nc.vector.tensor_mul(mu2[:], gsb[:, 0:1], gsb[:, 0:1])
                    av = small.tile([8, 1], F32, tag="small", name="av")
                    nc.vector.tensor_sub(av[:], gsb[:, 1:2], mu2[:])
                    nc.vector.tensor_scalar(out=av[:], in0=av[:], scalar1=EPS,
                                            scalar2=None, op0=ADD)
                    yi = small.tile([8, 1], I32, tag="small", name="yi")
                    nc.vector.tensor_scalar(out=yi[:], in0=av[:].bitcast(I32),
                                            scalar1=1, scalar2=None, op0=SHR)
                    nc.vector.tensor_sub(yi[:], magic_t[:], yi[:])
                    y = yi[:].bitcast(F32)
                    ah = small.tile([8, 1], F32, tag="small", name="ah")
                    nc.vector.tensor_scalar(out=ah[:], in0=av[:], scalar1=0.5,
                                            scalar2=None, op0=MUL)
                    t2 = small.tile([8, 1], F32, tag="small", name="t2")
                    nc.vector.tensor_mul(t2[:], y, y)
                    nc.vector.tensor_mul(t2[:], t2[:], ah[:])
                    nc.vector.tensor_scalar(out=t2[:], in0=t2[:],
                                            scalar1=-1.0, scalar2=1.5,
                                            op0=MUL, op1=ADD)
                    nc.vector.tensor_mul(y, y, t2[:])
                    grp = small.tile([8, 2], F32R, tag="small", name="grp")
                    nc.vector.tensor_copy(grp[:, 0:1], gsb[:, 0:1])
                    nc.vector.tensor_copy(grp[:, 1:2], y)
                    psc = psA.tile([128, 2], F32, tag="big", name="psc")
                    nc.tensor.matmul(psc[:], gt8_sb[:], grp[:],
                                     start=True, stop=True)
                    s_c = small.tile([128, 1], F32, tag="small", name="s_c")
                    nc.vector.tensor_mul(s_c[:], psc[:, 1:2], gnw_sb[:, k:k + 1])
                    t1 = small.tile([128, 1], F32, tag="small", name="t1")
                    nc.vector.tensor_mul(t1[:], psc[:, 0:1], s_c[:])
                    b_c = small.tile([128, 1], F32, tag="small", name="b_c")
                    nc.vector.tensor_sub(b_c[:], gnb_sb[:, k:k + 1], t1[:])
                    ht = singles.tile([128, T], BF16, tag=f"h{k}", name=f"h{k}")
                    for nb in range(NB):
                        sl = slice(512 * nb, 512 * (nb + 1))
                        if k == 2:
                            nc.vector.tensor_scalar(
                                out=ht[:, sl], in0=x_sb[k][:, sl],
                                scalar1=s_c[:], scalar2=b_c[:], op0=MUL,
                                op1=ADD)
                        else:
                            nc.scalar.activation(ht[:, sl], x_sb[k][:, sl],
                                                 IDENT, bias=b_c[:],
                                                 scale=s_c[:])
                    h_sb[k] = ht

                # ================= qk tiles =================
                qk_tiles = {}

                def gen_qk01():
                    # m = 0, 1 interleaved nb-major so QK(0) steps on the
                    # first t-half can start as early as possible
                    pqs = [psA.tile([128, T], F32, tag="big", name="pq")
                           for _ in range(2)]
                    for m in range(2):
                        qk_tiles[m] = singles.tile(
                            [128, T], BF16, tag=f"qk{m}{sfx}", name=f"qk{m}")
                    for nb in range(NB):
                        sl = slice(512 * nb, 512 * (nb + 1))
                        for m in range(2):
                            for i, k in enumerate(KORD):
                                nc.tensor.matmul(
                                    pqs[m][:, sl], wqk_ap(k, m),
                                    h_sb[k][:, sl], start=(i == 0),
                                    stop=(i == 3))
                        nc.scalar.activation(qk_tiles[0][:, sl], pqs[0][:, sl],
                                             IDENT, bias=bqk_sb[:, 0:1])
                        nc.vector.tensor_scalar(
                            out=qk_tiles[1][:, sl], in0=pqs[1][:, sl],
                            scalar1=bqk_sb[:, 1:2], scalar2=None, op0=ADD)

                gen_qk01()

                # ================= attention state =================
                ew_pairs = {}

                def _ew(h, j):
                    if (h, j) not in ew_pairs:
                        ew_pairs[(h, j)] = ewp.tile([128, 2, T], FP8, tag="ew",
                                                    name=f"ew{h}_{j}")
                    return ew_pairs[(h, j)]

                def emit_qk_step(h, st):
                    # logits for head h, s-tile st: 2 matmuls + 1 merged exp
                    p, slot = h // 2, h % 2
                    lo, hi = CH * slot, CH * (slot + 1)
                    ktile, qtile = qk_tiles[2 * p], qk_tiles[2 * p + 1]
                    j, pl = st // 2, st % 2
                    pw = psB.tile([128, T], F32, tag="pw", name="pw")
                    for nb in range(NB):
                        nc.tensor.matmul(
                            pw[:, 512 * nb:512 * (nb + 1)],
                            ktile[lo:hi, 128 * st:128 * (st + 1)],
                            qtile[lo:hi, 512 * nb:512 * (nb + 1)],
                            start=True, stop=True)
                    nc.scalar.activation(_ew(h, j)[:, pl, :], pw[:], EXP)

                def emit_qk_half_step(h, st, nb):
                    # one t-half of head h's logits (used to stretch head 7's
                    # exp stream over the last two windows)
                    p, slot = h // 2, h % 2
                    lo, hi = CH * slot, CH * (slot + 1)
                    ktile, qtile = qk_tiles[2 * p], qk_tiles[2 * p + 1]
                    j, pl = st // 2, st % 2
                    sl = slice(512 * nb, 512 * (nb + 1))
                    pw = psB.tile([128, 512], F32, tag="pw", name="pwh")
                    nc.tensor.matmul(
                        pw[:], ktile[lo:hi, 128 * st:128 * (st + 1)],
                        qtile[lo:hi, sl], start=True, stop=True)
                    nc.scalar.activation(_ew(h, j)[:, pl, sl], pw[:], EXP)

                # vt pair tiles (fp8, rider block cols 0:RID with col0 = ones)
                vt_sb = [singles.tile([128, 2, H, RID + CH], FP8,
                                      tag=f"vt{j}", name=f"vt{j}")
                         for j in range(NJ)]
                for j in range(NJ):
                    nc.vector.memset(vt_sb[j][:, :, :, 0:RID], 1.0)

                def emit_v_tile(st):
                    pv = psA.tile([128, C], F32, tag="big", name="pv")
                    for i, k in enumerate(KORD):
                        nc.tensor.matmul(pv[:],
                                         h_sb[k][:, 128 * st:128 * (st + 1)],
                                         wv_ap(k), start=(i == 0),
                                         stop=(i == 3))
                    nc.vector.tensor_copy(
                        vt_sb[st // 2][:, st % 2, :, RID:RID + CH],
                        pv[:].rearrange("p (h c) -> p h c", h=H))

                a_sb = [singles.tile([128, T], BF16, tag=f"a{p}",
                                     name=f"a{p}{sfx}") for p in range(NCHUNK)]
                acc_sb = [singles.tile([128, T], F32, tag=f"acc{m}",
                                       name=f"acc{m}{sfx}")
                          for m in range(NCHUNK)]

                # ---- prologue: v tiles + QK(0) steps (pv uses the big tag
                # ---- so the pw rotation stays a pure QK/exp double-buffer)
                for st in range(NT):
                    if st < 2:
                        emit_qk_half_step(0, st, 0)
                        emit_qk_half_step(0, st, 1)
                    else:
                        emit_qk_step(0, st)
                    emit_v_tile(st)

                # ================= duties =================
                def qk_spread_duty(m):
                    pq = psA.tile([128, T], F32, tag="big", name="pq")
                    qk = singles.tile([128, T], BF16, tag=f"qk{m}{sfx}",
                                      name=f"qk{m}")
                    qk_tiles[m] = qk
                    for nb in range(NB):
                        sl = slice(512 * nb, 512 * (nb + 1))
                        for i, k in enumerate(KORD):
                            nc.tensor.matmul(
                                pq[:, sl], wqk_ap(k, m), h_sb[k][:, sl],
                                start=(i == 0), stop=(i == 3))
                            yield
                        nc.vector.tensor_scalar(
                            out=qk[:, sl], in0=pq[:, sl],
                            scalar1=bqk_sb[:, m:m + 1], scalar2=None, op0=ADD)
                    yield

                def proj01_duty():
                    # acc[m] = (Wp0 @ a0 + Wp1 @ a1 + bproj) + x
                    for m in range(NCHUNK):
                        po = psA.tile([128, T], F32, tag="big", name="po")
                        for nb in range(NB):
                            sl = slice(512 * nb, 512 * (nb + 1))
                            nc.tensor.matmul(
                                po[:, sl], wproj_ap(0, m), a_sb[0][:, sl],
                                start=True, stop=False)
                            yield
                            nc.tensor.matmul(
                                po[:, sl], wproj_ap(1, m), a_sb[1][:, sl],
                                start=False, stop=True)
                            yield
                        nc.vector.scalar_tensor_tensor(
                            out=acc_sb[m][:], in0=po[:],
                            scalar=bproj_sb[:, m:m + 1], in1=x_sb[m][:],
                            op0=ADD, op1=ADD)
                        yield

                def proj2_duty(mlo, mhi):
                    # acc[m] += Wp2 @ a2
                    for m in range(mlo, mhi):
                        po = psA.tile([128, T], F32, tag="big", name="po")
                        for nb in range(NB):
                            sl = slice(512 * nb, 512 * (nb + 1))
                            nc.tensor.matmul(
                                po[:, sl], wproj_ap(2, m), a_sb[2][:, sl],
                                start=True, stop=True)
                            yield
                        nc.vector.tensor_add(acc_sb[m][:], po[:], acc_sb[m][:])
                        yield

                # ================= head loop =================
                last_rep = rep == n_reps - 1
                for h in range(H):
                    p, slot = h // 2, h % 2
                    lo, hi = CH * slot, CH * (slot + 1)
                    duties = []
                    if h == 0:
                        duties.append(qk_spread_duty(2))
                        duties.append(qk_spread_duty(3))
                    elif h == 1:
                        duties.append(qk_spread_duty(4))
                    elif h == 2:
                        duties.append(qk_spread_duty(5))
                    elif h == 3:
                        duties.append(qk_spread_duty(6))
                    elif h == 4:
                        duties.append(qk_spread_duty(7))
                    elif h == 5:
                        duties.append(proj01_duty())
                    elif h == 6:
                        duties.append(proj2_duty(0, 4))
                    pa = psA.tile([128, T], F32, tag="big", name="pa")
                    # feed ACT before the AV burst: first two qk steps
                    if h < 7:
                        emit_qk_step(h + 1, 0)
                        emit_qk_step(h + 1, 1)
                    # ew for this head is complete: AV burst, then normalize
                    for j in range(NJ):
                        ewt = ew_pairs.pop((h, j))
                        for nb in range(NB):
                            nc.tensor.matmul(
                                pa[:, 512 * nb:512 * (nb + 1)],
                                vt_sb[j][:, :, h, :],
                                ewt[:, :, 512 * nb:512 * (nb + 1)],
                                start=(j == 0), stop=(j == NJ - 1),
                                perf_mode=DR)
                    if h == H - 1:
                        # straight-line tail: normalize + pair-3 proj + out
                        for nb in range(NB):
                            sl = slice(512 * nb, 512 * (nb + 1))
                            rcb = recp.tile([CH, 512], F32, tag="rcb",
                                            name="rcb")
                            nc.vector.reciprocal_approx_fast(rcb[:],
                                                             pa[0:CH, sl])
                            nc.vector.tensor_mul(
                                a_sb[3][CH:128, sl], pa[RID:RID + CH, sl],
                                rcb[:])
                        for nb in range(NB):
                            sl = slice(512 * nb, 512 * (nb + 1))
                            for m in range(NCHUNK):
                                po = psA.tile([128, 512], F32, tag="big",
                                              name="pot")
                                nc.tensor.matmul(po[:], wproj_ap(3, m),
                                                 a_sb[3][:, sl],
                                                 start=True, stop=True)
                                nc.vector.tensor_add(acc_sb[m][:, sl], po[:],
                                                     acc_sb[m][:, sl])
                                if last_rep:
                                    nc.sync.dma_start(
                                        out_d[128 * m:128 * (m + 1), sl],
                                        acc_sb[m][:, sl])
                        continue
                    rec = recp.tile([CH, T], F32, tag="rec", name="rec")
                    nc.vector.reciprocal_approx_fast(rec[:], pa[0:CH, :])
                    nc.vector.tensor_mul(a_sb[p][lo:hi, :],
                                         pa[RID:RID + CH, :], rec[:])
                    for st in range(2, NT):
                        emit_qk_step(h + 1, st)
                        advanced = 0
                        while duties and advanced < 2:
                            try:
                                next(duties[0])
                                advanced += 1
                            except StopIteration:
                                duties.pop(0)
                    for g in duties:
                        for _ in g:
                            pass

    nc.compile()
    return nc


def _get_program(n_reps=1):
    key = ("prog", n_reps)
    if key not in _CACHE:
        _CACHE[key] = _build_program(n_reps)
    return _CACHE[key]


def kernel(x, gn_w, gn_b, qkv_w, qkv_b, proj_w, proj_b, _n_reps=1):
    x = np.asarray(x, dtype=np.float32)
    hw = _host_weights(np.asarray(gn_w, np.float32), np.asarray(gn_b, np.float32),
                       np.asarray(qkv_w, np.float32), np.asarray(qkv_b, np.float32),
                       np.asarray(proj_w, np.float32), np.asarray(proj_b, np.float32))
    xr = np.ascontiguousarray(x.reshape(B, C, T))
    nc = _get_program(_n_reps)
    in_maps = [dict(hw, x=xr[b]) for b in range(B)]
    res = run_bass_kernel_spmd(nc, in_maps, core_ids=list(range(B)))
    out = np.stack([res.results[b]["out"] for b in range(B)])
    return out.reshape(B, C, HS, WS).astype(np.float32)


# revision 18
# speedup vs baseline: 1.0899x; 1.0059x over previous
"""Trainium2 Bass kernel for MultiHeadSelfAttention (GroupNorm + QKV + attention + proj + residual).

Problem shape (hardcoded): x [8, 512, 32, 32] fp32, 8 heads, 32 groups.
Sharding: data-parallel over batch B=8 across the 8 NeuronCores (one batch per core).

Per-core pipeline (T = 1024 positions, C = 512 channels, ch = 64 per head):
  1. GroupNorm(32) chunk-pipelined: groups never cross a 128-channel chunk;
     rsqrt(var+eps) via the quake bit-hack + 2 Newton steps on DVE so the
     Activation engine runs softmax exps only (no act-table swaps).
  2. qkv = qkv_w @ h with host-reordered bf16 weights:
       - q,k tiles [128, T]: m-tile 2p = [k_h(2p)|k_h(2p+1)], 2p+1 = [q...]
       - v produced transposed per s-tile, packed as fp8e4 pairs for DoubleRow
  3. Per head: logits via PE (bf16), ONE merged exp [128,1024] per s-tile on
     ACT writing fp8e4 straight to SBUF; attention @ V via fp8 DoubleRow
     matmuls (two s-planes per instruction; 64-col rider block carries the
     softmax denominator in partition 0); DVE reciprocal + gpsimd
     partition_broadcast + DVE mul to normalize.
  4. proj accumulated in PSUM per pair-group; v-bias folded into the proj
     bias on the host; bias+residual fused via scalar_tensor_tensor. Only
     the head-7 contraction half remains for the tail.

All input DMAs ride one ordered SP queue (x + wqk first) so the first
softmax exp lands as early as possible; the exp stream is the critical
resource and runs back-to-back for the rest of the kernel.
"""

import ml_dtypes
import numpy as np

import concourse.bass as bass
import concourse.bacc as bacc
import concourse.tile as tile
import concourse.mybir as mybir
from concourse import library_config
from concourse.bass_utils import run_bass_kernel_spmd

B, C, HS, WS = 8, 512, 32, 32
T = HS * WS            # 1024
H = 8                  # heads
CH = C // H            # 64
G = 32                 # groups
CPG = C // G           # 16 channels per group
EPS = 1e-5
NCHUNK = C // 128      # 4 channel chunks
NT = T // 128          # 8 sequence tiles
NB = T // 512          # 2 psum banks over T
NJ = NT // 2           # 4 s-tile pairs (DoubleRow planes)
RID = 64               # rider cols per head (col 0 = ones); out partitions 128
MAGIC = 0x5F3759DF     # quake rsqrt seed
F32 = mybir.dt.float32
F32R = mybir.dt.float32r
I32 = mybir.dt.int32
BF16 = mybir.dt.bfloat16
FP8 = mybir.dt.float8e4
EXP = mybir.ActivationFunctionType.Exp
IDENT = mybir.ActivationFunctionType.Identity
DR = mybir.MatmulPerfMode.DoubleRow
MUL = mybir.AluOpType.mult
ADD = mybir.AluOpType.add
SHR = mybir.AluOpType.logical_shift_right

_CACHE = {}
KORD = (0, 1, 2, 3)


def _orig_row(kind, h, i):
    off = {"q": 0, "k": CH, "v": 2 * CH}[kind]
    return 192 * h + off + i


def _host_weights(gn_w, gn_b, qkv_w, qkv_b, proj_w, proj_b):
    scale2 = 1.0 / np.sqrt(CH)  # ch**-0.25 on both q and k -> fold into k
    rows = np.zeros(2 * C, dtype=np.int64)
    colscale = np.ones(2 * C, dtype=np.float32)
    for p in range(H // 2):
        for slot in range(2):
            h = 2 * p + slot
            for i in range(CH):
                col_k = (2 * p) * 128 + slot * CH + i
                rows[col_k] = _orig_row("k", h, i)
                colscale[col_k] = scale2
                col_q = (2 * p + 1) * 128 + slot * CH + i
                rows[col_q] = _orig_row("q", h, i)
    wqk = (qkv_w[rows, :] * colscale[:, None]).T.copy()      # [512, 1024]
    # two DMA tiles: chunks (0,1) and (2,3) side by side
    wqk_t = np.ascontiguousarray(
        wqk.reshape(2, 2, 128, 2 * C).transpose(0, 2, 1, 3).reshape(
            2, 128, 4 * C)).astype(ml_dtypes.bfloat16)
    bqk = (qkv_b[rows] * colscale).reshape(8, 128).T.copy()  # [128, 8]

    vrows = np.array([_orig_row("v", h, i) for h in range(H) for i in range(CH)])
    wv = qkv_w[vrows, :].T.copy()                            # [512, 512] (c, c_v)
    wv_t = np.ascontiguousarray(
        wv.reshape(NCHUNK, 128, C).transpose(1, 0, 2).reshape(
            128, NCHUNK * C)).astype(ml_dtypes.bfloat16)     # [128, 2048]

    bv = qkv_b[vrows]
    bproj_full = proj_b + proj_w @ bv                        # [512]
    wproj = proj_w.T.copy()                                  # [512(c), 512(o)]
    wproj_t = np.ascontiguousarray(
        wproj.reshape(NCHUNK, 128, C).transpose(1, 0, 2).reshape(
            128, NCHUNK * C)).astype(ml_dtypes.bfloat16)

    # consolidated f32 consts [128, 24]: g8 | gnw | gnb | bqk | bproj
    g8 = np.zeros((128, 8), dtype=np.float32)
    gt8 = np.zeros((8, 128), dtype=np.float32)
    for u in range(128):
        g8[u, u // CPG] = 1.0 / CPG
        gt8[u // CPG, u] = 1.0
    cst = np.concatenate([
        g8,
        gn_w.reshape(NCHUNK, 128).T,
        gn_b.reshape(NCHUNK, 128).T,
        bqk,
        bproj_full.reshape(NCHUNK, 128).T,
    ], axis=1).astype(np.float32)                            # [128, 28]
    return {"cst": cst, "gt8": gt8, "wqk": wqk_t, "wv": wv_t,
            "wproj": wproj_t}


def _build_program(n_reps=1, ew_bufs=12):
    nc = bacc.Bacc("TRN2", target_bir_lowering=False, debug=False, num_devices=8)
    dt_in = [
        ("x", [C, T], F32), ("cst", [128, 28], F32R), ("gt8", [8, 128], F32R),
        ("wqk", [2, 128, 4 * C], BF16), ("wv", [128, NCHUNK * C], BF16),
        ("wproj", [128, NCHUNK * C], BF16),
    ]
    d = {name: nc.dram_tensor(name, shape, dt, kind="ExternalInput").ap()
         for name, shape, dt in dt_in}
    out_d = nc.dram_tensor("out", [C, T], F32, kind="ExternalOutput").ap()

    with tile.TileContext(nc) as tc:
        with (
            tc.tile_pool(name="singles", bufs=1) as singles,
            tc.tile_pool(name="small", bufs=16) as small,
            tc.tile_pool(name="ewp", bufs=ew_bufs) as ewp,
            tc.tile_pool(name="recp", bufs=2) as recp,
            tc.tile_pool(name="psA", bufs=2, space="PSUM") as psA,
            tc.tile_pool(name="psB", bufs=2, space="PSUM") as psB,
        ):
            # ---- one ordered DMA stream on the SP queue: consts, then x
            # ---- halves interleaved with wqk, then wv/wproj ----
            cst = singles.tile([128, 28], F32R, tag="cst", name="cst")
            nc.sync.dma_start(cst[:], d["cst"][:])
            gt8_sb = singles.tile([8, 128], F32R, tag="gt8", name="gt8")
            nc.sync.dma_start(gt8_sb[:], d["gt8"][:])
            g8_sb = cst[:, 0:8]
            gnw_sb = cst[:, 8:12].bitcast(F32)
            gnb_sb = cst[:, 12:16].bitcast(F32)
            bqk_sb = cst[:, 16:24].bitcast(F32)
            bproj_sb = cst[:, 24:28].bitcast(F32)

            x_sb = [singles.tile([128, T], F32, tag=f"x{k}", name=f"x{k}")
                    for k in range(NCHUNK)]
            wqk_sb = [singles.tile([128, 4 * C], BF16, tag=f"wqk{g}",
                                   name=f"wqk{g}") for g in range(2)]
            for k in range(NCHUNK):
                for nb in range(NB):
                    sl = slice(512 * nb, 512 * (nb + 1))
                    nc.sync.dma_start(x_sb[k][:, sl],
                                      d["x"][128 * k:128 * (k + 1), sl])
            for g in range(2):
                nc.sync.dma_start(wqk_sb[g][:], d["wqk"][g])
            wv_sb = singles.tile([128, NCHUNK * C], BF16, tag="wv", name="wv")
            nc.sync.dma_start(wv_sb[:], d["wv"][:])
            wproj_sb = singles.tile([128, NCHUNK * C], BF16, tag="wproj",
                                    name="wproj")
            nc.sync.dma_start(wproj_sb[:], d["wproj"][:])

            def wqk_ap(k, m):
                # chunk k, m-tile column block [128, 128]
                return wqk_sb[k // 2][:, 1024 * (k % 2) + 128 * m:
                                      1024 * (k % 2) + 128 * (m + 1)]

            def wv_ap(k):
                return wv_sb[:, 512 * k:512 * (k + 1)]

            def wproj_ap(p, m, clo=0, chi=128):
                return wproj_sb[clo:chi, 512 * p + 128 * m:512 * p + 128 * (m + 1)]

            magic_t = singles.tile([8, 1], I32, tag="magic", name="magic")
            nc.vector.memset(magic_t[:], MAGIC)
            # prime the exp/identity activation table while ACT is idle
            prim = singles.tile([1, 1], F32, tag="prim", name="prim")
            nc.vector.memset(prim[:], 0.0)
            nc.scalar.activation(prim[:], prim[:], IDENT)

            for rep in range(n_reps):
                sfx = f"r{rep}"
                # ================= GroupNorm (per chunk) =================
                h_sb = [None] * NCHUNK
                stats_t = {}
                # phase 1: per-channel stats -- chunks 1-3 on DVE (bn_stats),
                # chunk 0 via ACT accumulators
                for k in (1, 2, 3):
                    stats = small.tile([128, 2], F32R, tag="small", name="stats")
                    st6 = small.tile([128, 2, 6], F32, tag="small", name="st6")
                    nc.vector.bn_stats(st6[:, 0, :], x_sb[k][:, 0:512])
                    nc.vector.bn_stats(st6[:, 1, :], x_sb[k][:, 512:1024])
                    mv = small.tile([128, 2], F32, tag="small", name="mv")
                    nc.vector.bn_aggr(mv[:], st6[:])
                    m2 = small.tile([128, 1], F32, tag="small", name="m2")
                    nc.vector.tensor_mul(m2[:], mv[:, 0:1], mv[:, 0:1])
                    nc.vector.tensor_copy(stats[:, 0:1], mv[:, 0:1])
                    nc.vector.tensor_add(stats[:, 1:2], mv[:, 1:2], m2[:])
                    stats_t[k] = stats
                for k in (0,):
                    stats = small.tile([128, 2], F32R, tag="small", name="stats")
                    scr = small.tile([128, T], BF16, tag="gnscr", bufs=2,
                                     name="scr")
                    asm = small.tile([128, 1], F32, tag="small", name="asm")
                    nc.scalar.activation(scr[:], x_sb[k][:], IDENT,
                                         accum_out=asm[:])
                    asq = small.tile([128, 1], F32, tag="small", name="asq")
                    nc.scalar.activation(scr[:], x_sb[k][:],
                                         mybir.ActivationFunctionType.Square,
                                         accum_out=asq[:])
                    nc.vector.tensor_scalar(out=stats[:, 0:1], in0=asm[:],
                                            scalar1=1.0 / T, scalar2=None,
                                            op0=MUL)
                    nc.vector.tensor_scalar(out=stats[:, 1:2], in0=asq[:],
                                            scalar1=1.0 / T, scalar2=None,
                                            op0=MUL)
                    stats_t[k] = stats
                # phase 2: batched group combine for all four chunks at once
                gall = small.tile([8, 2, NCHUNK], F32, tag="small", name="gall")
                for k in range(NCHUNK):
                    psg = psA.tile([8, 2], F32, tag="big", name="psg")
                    nc.tensor.matmul(psg[:], g8_sb, stats_t[k][:],
                                     start=True, stop=True)
                    nc.vector.tensor_copy(gall[:, :, k], psg[:])
                mu_a = gall[:, 0, :]          # [8, 4] group means
                ex_a = gall[:, 1, :]          # [8, 4] group E[x^2]
                mu2a = small.tile([8, NCHUNK], F32, tag="small", name="mu2a")
                nc.vector.tensor_mul(mu2a[:], mu_a, mu_a)
                ava = small.tile([8, NCHUNK], F32, tag="small", name="ava")
                nc.vector.scalar_tensor_tensor(
                    out=ava[:], in0=ex_a, scalar=EPS, in1=mu2a[:],
                    op0=ADD, op1=mybir.AluOpType.subtract)
                yia = small.tile([8, NCHUNK], I32, tag="small", name="yia")
                nc.vector.tensor_scalar(out=yia[:], in0=ava[:].bitcast(I32),
                                        scalar1=1, scalar2=None, op0=SHR)
                mga = small.tile([8, NCHUNK], I32, tag="small", name="mga")
                nc.vector.memset(mga[:], MAGIC)
                nc.vector.tensor_sub(yia[:], mga[:], yia[:])
                ya = yia[:].bitcast(F32)
                t2a = small.tile([8, NCHUNK], F32, tag="small", name="t2a")
                nc.vector.tensor_mul(t2a[:], ya, ya)
                nc.vector.tensor_mul(t2a[:], t2a[:], ava[:])
                nc.vector.tensor_scalar(out=t2a[:], in0=t2a[:], scalar1=-0.5,
                                        scalar2=1.5, op0=MUL, op1=ADD)
                grpa = small.tile([8, 2, NCHUNK], F32R, tag="small", name="grpa")
                nc.vector.tensor_copy(grpa[:, 0, :], mu_a)
                nc.vector.tensor_mul(grpa[:, 1, :], ya, t2a[:])
                psca = psA.tile([128, 2 * NCHUNK], F32, tag="big", name="psca")
                nc.tensor.matmul(psca[:], gt8_sb[:],
                                 grpa[:].rearrange("g a k -> g (a k)"),
                                 start=True, stop=True)
                s_a = small.tile([128, NCHUNK], F32, tag="small", name="s_a")
                nc.vector.tensor_mul(s_a[:], psca[:, NCHUNK:2 * NCHUNK],
                                     gnw_sb)
                t1a = small.tile([128, NCHUNK], F32, tag="small", name="t1a")
                nc.vector.tensor_mul(t1a[:], psca[:, 0:NCHUNK], s_a[:])
                b_a = small.tile([128, NCHUNK], F32, tag="small", name="b_a")
                nc.vector.tensor_sub(b_a[:], gnb_sb, t1a[:])
                # affine: nb0 halves first (unblocks the first qk tiles)
                for k in range(NCHUNK):
                    h_sb[k] = singles.tile([128, T], BF16, tag=f"h{k}",
                                           name=f"h{k}")
                for nb in range(NB):
                    sl = slice(512 * nb, 512 * (nb + 1))
                    for k in range(NCHUNK):
                        if k == 2:
                            nc.vector.tensor_scalar(
                                out=h_sb[k][:, sl], in0=x_sb[k][:, sl],
                                scalar1=s_a[:, k:k + 1],
                                scalar2=b_a[:, k:k + 1], op0=MUL, op1=ADD)
                        else:
                            nc.scalar.activation(h_sb[k][:, sl],
                                                 x_sb[k][:, sl], IDENT,
                                                 bias=b_a[:, k:k + 1],
                                                 scale=s_a[:, k:k + 1])

                # ================= qk tiles =================
                qk_tiles = {}

                def gen_qk01():
                    # m = 0, 1 interleaved nb-major so QK(0) steps on the
                    # first t-half can start as early as possible
                    pqs = [psA.tile([128, T], F32, tag="big", name="pq")
                           for _ in range(2)]
                    for m in range(2):
                        qk_tiles[m] = singles.tile(
                            [128, T], BF16, tag=f"qk{m}{sfx}", name=f"qk{m}")
                    for nb in range(NB):
                        sl = slice(512 * nb, 512 * (nb + 1))
                        for m in range(2):
                            for i, k in enumerate(KORD):
                                nc.tensor.matmul(
                                    pqs[m][:, sl], wqk_ap(k, m),
                                    h_sb[k][:, sl], start=(i == 0),
                                    stop=(i == 3))
                        nc.scalar.activation(qk_tiles[0][:, sl], pqs[0][:, sl],
                                             IDENT, bias=bqk_sb[:, 0:1])
                        nc.vector.tensor_scalar(
                            out=qk_tiles[1][:, sl], in0=pqs[1][:, sl],
                            scalar1=bqk_sb[:, 1:2], scalar2=None, op0=ADD)

                gen_qk01()

                # ================= attention state =================
                ew_pairs = {}

                def _ew(h, j):
                    if (h, j) not in ew_pairs:
                        ew_pairs[(h, j)] = ewp.tile([128, 2, T], FP8, tag="ew",
                                                    name=f"ew{h}_{j}")
                    return ew_pairs[(h, j)]

                def emit_qk_step(h, st):
                    # logits for head h, s-tile st: 2 matmuls + 1 merged exp
                    p, slot = h // 2, h % 2
                    lo, hi = CH * slot, CH * (slot + 1)
                    ktile, qtile = qk_tiles[2 * p], qk_tiles[2 * p + 1]
                    j, pl = st // 2, st % 2
                    pw = psB.tile([128, T], F32, tag="pw", name="pw")
                    for nb in range(NB):
                        nc.tensor.matmul(
                            pw[:, 512 * nb:512 * (nb + 1)],
                            ktile[lo:hi, 128 * st:128 * (st + 1)],
                            qtile[lo:hi, 512 * nb:512 * (nb + 1)],
                            start=True, stop=True)
                    nc.scalar.activation(_ew(h, j)[:, pl, :], pw[:], EXP)

                def emit_qk_half_step(h, st, nb):
                    # one t-half of head h's logits (used to stretch head 7's
                    # exp stream over the last two windows)
                    p, slot = h // 2, h % 2
                    lo, hi = CH * slot, CH * (slot + 1)
                    ktile, qtile = qk_tiles[2 * p], qk_tiles[2 * p + 1]
                    j, pl = st // 2, st % 2
                    sl = slice(512 * nb, 512 * (nb + 1))
                    pw = psB.tile([128, 512], F32, tag="pw", name="pwh")
                    nc.tensor.matmul(
                        pw[:], ktile[lo:hi, 128 * st:128 * (st + 1)],
                        qtile[lo:hi, sl], start=True, stop=True)
                    nc.scalar.activation(_ew(h, j)[:, pl, sl], pw[:], EXP)

                # vt pair tiles (fp8, rider block cols 0:RID with col0 = ones)
                vt_sb = [singles.tile([128, 2, H, RID + CH], FP8,
                                      tag=f"vt{j}", name=f"vt{j}")
                         for j in range(NJ)]
                for j in range(NJ):
                    nc.vector.memset(vt_sb[j][:, :, :, 0:RID], 1.0)

                def emit_v_tile(st):
                    pv = psA.tile([128, C], F32, tag="big", name="pv")
                    for i, k in enumerate(KORD):
                        nc.tensor.matmul(pv[:],
                                         h_sb[k][:, 128 * st:128 * (st + 1)],
                                         wv_ap(k), start=(i == 0),
                                         stop=(i == 3))
                    nc.vector.tensor_copy(
                        vt_sb[st // 2][:, st % 2, :, RID:RID + CH],
                        pv[:].rearrange("p (h c) -> p h c", h=H))

                a_sb = [singles.tile([128, T], BF16, tag=f"a{p}",
                                     name=f"a{p}{sfx}") for p in range(NCHUNK)]
                acc_sb = [singles.tile([128, T], F32, tag=f"acc{m}",
                                       name=f"acc{m}{sfx}")
                          for m in range(NCHUNK)]

                # ---- prologue: v tiles + QK(0) steps (pv uses the big tag
                # ---- so the pw rotation stays a pure QK/exp double-buffer)
                for st in range(NT):
                    if st < 2:
                        emit_qk_half_step(0, st, 0)
                        emit_qk_half_step(0, st, 1)
                    else:
                        emit_qk_step(0, st)
                    emit_v_tile(st)

                # ================= duties =================
                def qk_spread_duty(m):
                    pq = psA.tile([128, T], F32, tag="big", name="pq")
                    qk = singles.tile([128, T], BF16, tag=f"qk{m}{sfx}",
                                      name=f"qk{m}")
                    qk_tiles[m] = qk
                    for nb in range(NB):
                        sl = slice(512 * nb, 512 * (nb + 1))
                        for i, k in enumerate(KORD):
                            nc.tensor.matmul(
                                pq[:, sl], wqk_ap(k, m), h_sb[k][:, sl],
                                start=(i == 0), stop=(i == 3))
                            yield
                        nc.vector.tensor_scalar(
                            out=qk[:, sl], in0=pq[:, sl],
                            scalar1=bqk_sb[:, m:m + 1], scalar2=None, op0=ADD)
                    yield

                def proj01_duty():
                    # acc[m] = (Wp0 @ a0 + Wp1 @ a1 + bproj) + x
                    for m in range(NCHUNK):
                        po = psA.tile([128, T], F32, tag="big", name="po")
                        for nb in range(NB):
                            sl = slice(512 * nb, 512 * (nb + 1))
                            nc.tensor.matmul(
                                po[:, sl], wproj_ap(0, m), a_sb[0][:, sl],
                                start=True, stop=False)
                            yield
                            nc.tensor.matmul(
                                po[:, sl], wproj_ap(1, m), a_sb[1][:, sl],
                                start=False, stop=True)
                            yield
                        nc.vector.scalar_tensor_tensor(
                            out=acc_sb[m][:], in0=po[:],
                            scalar=bproj_sb[:, m:m + 1], in1=x_sb[m][:],
                            op0=ADD, op1=ADD)
                        yield

                def proj2_duty(mlo, mhi):
                    # acc[m] += Wp2 @ a2
                    for m in range(mlo, mhi):
                        po = psA.tile([128, T], F32, tag="big", name="po")
                        for nb in range(NB):
                            sl = slice(512 * nb, 512 * (nb + 1))
                            nc.tensor.matmul(
                                po[:, sl], wproj_ap(2, m), a_sb[2][:, sl],
                                start=True, stop=True)
                            yield
                        nc.vector.tensor_add(acc_sb[m][:], po[:], acc_sb[m][:])
                        yield

                # ================= head loop =================
                last_rep = rep == n_reps - 1

                def tail_nb(nb, pa7):
                    # normalize head 7's nb half, then pair-3 proj + out
                    sl = slice(512 * nb, 512 * (nb + 1))
                    rcb = recp.tile([CH, 512], F32, tag="rcb", name="rcb")
                    nc.vector.reciprocal_approx_fast(rcb[:], pa7[0:CH, sl])
                    nc.vector.tensor_mul(a_sb[3][CH:128, sl],
                                         pa7[RID:RID + CH, sl], rcb[:])
                    yield
                    for m in range(NCHUNK):
                        po = psA.tile([128, 512], F32, tag="big", name="pot")
                        nc.tensor.matmul(po[:], wproj_ap(3, m),
                                         a_sb[3][:, sl], start=True, stop=True)
                        nc.vector.tensor_add(acc_sb[m][:, sl], po[:],
                                             acc_sb[m][:, sl])
                        if last_rep:
                            nc.sync.dma_start(out_d[128 * m:128 * (m + 1), sl],
                                              acc_sb[m][:, sl])
                        yield

                pa7 = None
                for h in range(H - 1):
                    p, slot = h // 2, h % 2
                    lo, hi = CH * slot, CH * (slot + 1)
                    duties = []
                    if h == 0:
                        duties.append(qk_spread_duty(2))
                        duties.append(qk_spread_duty(3))
                    elif h == 1:
                        duties.append(qk_spread_duty(4))
                        duties.append(qk_spread_duty(5))
                    elif h == 2:
                        duties.append(qk_spread_duty(6))
                        duties.append(qk_spread_duty(7))
                    elif h == 4:
                        duties.append(proj01_duty())
                    elif h == 5:
                        duties.append(proj2_duty(0, 4))
                    pa = psA.tile([128, T], F32, tag="big", name="pa")
                    # feed ACT before the AV burst: first two qk steps
                    if h < 6:
                        emit_qk_step(h + 1, 0)
                        emit_qk_step(h + 1, 1)
                    else:
                        emit_qk_half_step(7, 0, 0)
                        emit_qk_half_step(7, 1, 0)
                    # ew for this head is complete: AV burst, then normalize
                    for j in range(NJ):
                        ewt = ew_pairs.pop((h, j))
                        for nb in range(NB):
                            nc.tensor.matmul(
                                pa[:, 512 * nb:512 * (nb + 1)],
                                vt_sb[j][:, :, h, :],
                                ewt[:, :, 512 * nb:512 * (nb + 1)],
                                start=(j == 0), stop=(j == NJ - 1),
                                perf_mode=DR)
                    rec = recp.tile([CH, T], F32, tag="rec", name="rec")
                    nc.vector.reciprocal_approx_fast(rec[:], pa[0:CH, :])
                    nc.vector.tensor_mul(a_sb[p][lo:hi, :],
                                         pa[RID:RID + CH, :], rec[:])
                    if h < 6:
                        for st in range(2, NT):
                            emit_qk_step(h + 1, st)
                            advanced = 0
                            while duties and advanced < 2:
                                try:
                                    next(duties[0])
                                    advanced += 1
                                except StopIteration:
                                    duties.pop(0)
                    else:
                        # window 6 also carries head 7: nb0 halves first, so
                        # the nb0 attention/proj tail overlaps the exp stream
                        halves = ([(st, 0) for st in range(2, NT)]
                                  + [(st, 1) for st in range(NT)])
                        pa7 = psA.tile([128, T], F32, tag="big", name="pa7")
                        for step in range(NT):
                            for _ in range(2):
                                if halves:
                                    s7, n7 = halves.pop(0)
                                    emit_qk_half_step(7, s7, n7)
                            if step == 3:
                                # all nb0 logits emitted: head-7 nb0 AV
                                for j in range(NJ):
                                    nc.tensor.matmul(
                                        pa7[:, 0:512], vt_sb[j][:, :, 7, :],
                                        ew_pairs[(7, j)][:, :, 0:512],
                                        start=(j == 0), stop=(j == NJ - 1),
                                        perf_mode=DR)
                                duties.append(tail_nb(0, pa7))
                            advanced = 0
                            while duties and advanced < 2:
                                try:
                                    next(duties[0])
                                    advanced += 1
                                except StopIteration:
                                    duties.pop(0)
                    for g in duties:
                        for _ in g:
                            pass

                # ====== tail: head-7 nb1 attention + proj + out ======
                for j in range(NJ):
                    ewt = ew_pairs.pop((7, j))
                    nc.tensor.matmul(
                        pa7[:, 512:1024], vt_sb[j][:, :, 7, :],
                        ewt[:, :, 512:1024], start=(j == 0),
                        stop=(j == NJ - 1), perf_mode=DR)
                for _ in tail_nb(1, pa7):
                    pass

    nc.compile()
    return nc


def _get_program(n_reps=1):
    key = ("prog", n_reps)
    if key not in _CACHE:
        _CACHE[key] = _build_program(n_reps)
    return _CACHE[key]


def kernel(x, gn_w, gn_b, qkv_w, qkv_b, proj_w, proj_b, _n_reps=1):
    x = np.asarray(x, dtype=np.float32)
    hw = _host_weights(np.asarray(gn_w, np.float32), np.asarray(gn_b, np.float32),
                       np.asarray(qkv_w, np.float32), np.asarray(qkv_b, np.float32),
                       np.asarray(proj_w, np.float32), np.asarray(proj_b, np.float32))
    xr = np.ascontiguousarray(x.reshape(B, C, T))
    nc = _get_program(_n_reps)
    in_maps = [dict(hw, x=xr[b]) for b in range(B)]
    res = run_bass_kernel_spmd(nc, in_maps, core_ids=list(range(B)))
    out = np.stack([res.results[b]["out"] for b in range(B)])
    return out.reshape(B, C, HS, WS).astype(np.float32)


# revision 20
# speedup vs baseline: 1.0911x; 1.0010x over previous
"""Trainium2 Bass kernel for MultiHeadSelfAttention (GroupNorm + QKV + attention + proj + residual).

Problem shape (hardcoded): x [8, 512, 32, 32] fp32, 8 heads, 32 groups.
Sharding: data-parallel over batch B=8 across the 8 NeuronCores (one batch per core).

Per-core pipeline (T = 1024 positions, C = 512 channels, ch = 64 per head):
  1. GroupNorm(32) chunk-pipelined: groups never cross a 128-channel chunk;
     rsqrt(var+eps) via the quake bit-hack + 2 Newton steps on DVE so the
     Activation engine runs softmax exps only (no act-table swaps).
  2. qkv = qkv_w @ h with host-reordered bf16 weights:
       - q,k tiles [128, T]: m-tile 2p = [k_h(2p)|k_h(2p+1)], 2p+1 = [q...]
       - v produced transposed per s-tile, packed as fp8e4 pairs for DoubleRow
  3. Per head: logits via PE (bf16), ONE merged exp [128,1024] per s-tile on
     ACT writing fp8e4 straight to SBUF; attention @ V via fp8 DoubleRow
     matmuls (two s-planes per instruction; 64-col rider block carries the
     softmax denominator in partition 0); DVE reciprocal + gpsimd
     partition_broadcast + DVE mul to normalize.
  4. proj accumulated in PSUM per pair-group; v-bias folded into the proj
     bias on the host; bias+residual fused via scalar_tensor_tensor. Only
     the head-7 contraction half remains for the tail.

All input DMAs ride one ordered SP queue (x + wqk first) so the first
softmax exp lands as early as possible; the exp stream is the critical
resource and runs back-to-back for the rest of the kernel.
"""

import ml_dtypes
import numpy as np

import concourse.bass as bass
import concourse.bacc as bacc
import concourse.tile as tile
import concourse.mybir as mybir
from concourse import library_config
from concourse.bass_utils import run_bass_kernel_spmd

B, C, HS, WS = 8, 512, 32, 32
T = HS * WS            # 1024
H = 8                  # heads
CH = C // H            # 64
G = 32                 # groups
CPG = C // G           # 16 channels per group
EPS = 1e-5
NCHUNK = C // 128      # 4 channel chunks
NT = T // 128          # 8 sequence tiles
NB = T // 512          # 2 psum banks over T
NJ = NT // 2           # 4 s-tile pairs (DoubleRow planes)
RID = 64               # rider cols per head (col 0 = ones); out partitions 128
MAGIC = 0x5F3759DF     # quake rsqrt seed
F32 = mybir.dt.float32
F32R = mybir.dt.float32r
I32 = mybir.dt.int32
BF16 = mybir.dt.bfloat16
FP8 = mybir.dt.float8e4
EXP = mybir.ActivationFunctionType.Exp
IDENT = mybir.ActivationFunctionType.Identity
DR = mybir.MatmulPerfMode.DoubleRow
MUL = mybir.AluOpType.mult
ADD = mybir.AluOpType.add
SHR = mybir.AluOpType.logical_shift_right

_CACHE = {}
KORD = (0, 1, 2, 3)


def _orig_row(kind, h, i):
    off = {"q": 0, "k": CH, "v": 2 * CH}[kind]
    return 192 * h + off + i


def _host_weights(gn_w, gn_b, qkv_w, qkv_b, proj_w, proj_b):
    scale2 = 1.0 / np.sqrt(CH)  # ch**-0.25 on both q and k -> fold into k
    rows = np.zeros(2 * C, dtype=np.int64)
    colscale = np.ones(2 * C, dtype=np.float32)
    for p in range(H // 2):
        for slot in range(2):
            h = 2 * p + slot
            for i in range(CH):
                col_k = (2 * p) * 128 + slot * CH + i
                rows[col_k] = _orig_row("k", h, i)
                colscale[col_k] = scale2
                col_q = (2 * p + 1) * 128 + slot * CH + i
                rows[col_q] = _orig_row("q", h, i)
    wqk = (qkv_w[rows, :] * colscale[:, None]).T.copy()      # [512, 1024]
    # two DMA tiles: chunks (0,1) and (2,3) side by side
    wqk_t = np.ascontiguousarray(
        wqk.reshape(2, 2, 128, 2 * C).transpose(0, 2, 1, 3).reshape(
            2, 128, 4 * C)).astype(ml_dtypes.bfloat16)
    bqk = (qkv_b[rows] * colscale).reshape(8, 128).T.copy()  # [128, 8]

    vrows = np.array([_orig_row("v", h, i) for h in range(H) for i in range(CH)])
    wv = qkv_w[vrows, :].T.copy()                            # [512, 512] (c, c_v)
    wv_t = np.ascontiguousarray(
        wv.reshape(NCHUNK, 128, C).transpose(1, 0, 2).reshape(
            128, NCHUNK * C)).astype(ml_dtypes.bfloat16)     # [128, 2048]

    bv = qkv_b[vrows]
    bproj_full = proj_b + proj_w @ bv                        # [512]
    wproj = proj_w.T.copy()                                  # [512(c), 512(o)]
    wproj_t = np.ascontiguousarray(
        wproj.reshape(NCHUNK, 128, C).transpose(1, 0, 2).reshape(
            128, NCHUNK * C)).astype(ml_dtypes.bfloat16)

    # consolidated f32 consts [128, 24]: g8 | gnw | gnb | bqk | bproj
    g8 = np.zeros((128, 8), dtype=np.float32)
    gt8 = np.zeros((8, 128), dtype=np.float32)
    for u in range(128):
        g8[u, u // CPG] = 1.0 / CPG
        gt8[u // CPG, u] = 1.0
    cst = np.concatenate([
        g8,
        gn_w.reshape(NCHUNK, 128).T,
        gn_b.reshape(NCHUNK, 128).T,
        bqk,
        bproj_full.reshape(NCHUNK, 128).T,
    ], axis=1).astype(np.float32)                            # [128, 28]
    return {"cst": cst, "gt8": gt8, "wqk": wqk_t, "wv": wv_t,
            "wproj": wproj_t}


def _build_program(n_reps=1, ew_bufs=12):
    nc = bacc.Bacc("TRN2", target_bir_lowering=False, debug=False, num_devices=8)
    dt_in = [
        ("x", [C, T], F32), ("cst", [128, 28], F32R), ("gt8", [8, 128], F32R),
        ("wqk", [2, 128, 4 * C], BF16), ("wv", [128, NCHUNK * C], BF16),
        ("wproj", [128, NCHUNK * C], BF16),
    ]
    d = {name: nc.dram_tensor(name, shape, dt, kind="ExternalInput").ap()
         for name, shape, dt in dt_in}
    out_d = nc.dram_tensor("out", [C, T], F32, kind="ExternalOutput").ap()

    with tile.TileContext(nc) as tc:
        with (
            tc.tile_pool(name="singles", bufs=1) as singles,
            tc.tile_pool(name="small", bufs=16) as small,
            tc.tile_pool(name="ewp", bufs=ew_bufs) as ewp,
            tc.tile_pool(name="recp", bufs=2) as recp,
            tc.tile_pool(name="psA", bufs=2, space="PSUM") as psA,
            tc.tile_pool(name="psB", bufs=2, space="PSUM") as psB,
        ):
            # ---- one ordered DMA stream on the SP queue: consts, then x
            # ---- halves interleaved with wqk, then wv/wproj ----
            cst = singles.tile([128, 28], F32R, tag="cst", name="cst")
            nc.sync.dma_start(cst[:], d["cst"][:])
            gt8_sb = singles.tile([8, 128], F32R, tag="gt8", name="gt8")
            nc.sync.dma_start(gt8_sb[:], d["gt8"][:])
            g8_sb = cst[:, 0:8]
            gnw_sb = cst[:, 8:12].bitcast(F32)
            gnb_sb = cst[:, 12:16].bitcast(F32)
            bqk_sb = cst[:, 16:24].bitcast(F32)
            bproj_sb = cst[:, 24:28].bitcast(F32)

            x_sb = [singles.tile([128, T], F32, tag=f"x{k}", name=f"x{k}")
                    for k in range(NCHUNK)]
            wqk_sb = [singles.tile([128, 4 * C], BF16, tag=f"wqk{g}",
                                   name=f"wqk{g}") for g in range(2)]
            for k in range(NCHUNK):
                for nb in range(NB):
                    sl = slice(512 * nb, 512 * (nb + 1))
                    nc.sync.dma_start(x_sb[k][:, sl],
                                      d["x"][128 * k:128 * (k + 1), sl])
            for g in range(2):
                nc.sync.dma_start(wqk_sb[g][:], d["wqk"][g])
            wv_sb = singles.tile([128, NCHUNK * C], BF16, tag="wv", name="wv")
            nc.sync.dma_start(wv_sb[:], d["wv"][:])
            wproj_sb = singles.tile([128, NCHUNK * C], BF16, tag="wproj",
                                    name="wproj")
            nc.sync.dma_start(wproj_sb[:], d["wproj"][:])

            def wqk_ap(k, m):
                # chunk k, m-tile column block [128, 128]
                return wqk_sb[k // 2][:, 1024 * (k % 2) + 128 * m:
                                      1024 * (k % 2) + 128 * (m + 1)]

            def wv_ap(k):
                return wv_sb[:, 512 * k:512 * (k + 1)]

            def wproj_ap(p, m, clo=0, chi=128):
                return wproj_sb[clo:chi, 512 * p + 128 * m:512 * p + 128 * (m + 1)]

            magic_t = singles.tile([8, 1], I32, tag="magic", name="magic")
            nc.vector.memset(magic_t[:], MAGIC)
            # prime the exp/identity activation table while ACT is idle
            prim = singles.tile([1, 1], F32, tag="prim", name="prim")
            nc.vector.memset(prim[:], 0.0)
            nc.scalar.activation(prim[:], prim[:], IDENT)

            for rep in range(n_reps):
                sfx = f"r{rep}"
                # ================= GroupNorm (per chunk) =================
                h_sb = [None] * NCHUNK
                stats_all = small.tile([128, 2, NCHUNK], F32R, tag="statsall",
                                       bufs=2, name="stats_all")
                # phase 1: per-channel stats -- chunks 1-3 on DVE (bn_stats),
                # chunk 0 via ACT accumulators
                for k in (1, 2, 3):
                    st6 = small.tile([128, 2, 6], F32, tag="small", name="st6")
                    nc.vector.bn_stats(st6[:, 0, :], x_sb[k][:, 0:512])
                    nc.vector.bn_stats(st6[:, 1, :], x_sb[k][:, 512:1024])
                    mv = small.tile([128, 2], F32, tag="small", name="mv")
                    nc.vector.bn_aggr(mv[:], st6[:])
                    m2 = small.tile([128, 1], F32, tag="small", name="m2")
                    nc.vector.tensor_mul(m2[:], mv[:, 0:1], mv[:, 0:1])
                    nc.vector.tensor_copy(stats_all[:, 0, k:k + 1],
                                          mv[:, 0:1])
                    nc.vector.tensor_add(stats_all[:, 1, k:k + 1],
                                         mv[:, 1:2], m2[:])
                for k in (0,):
                    scr = small.tile([128, T], BF16, tag="gnscr", bufs=2,
                                     name="scr")
                    asm = small.tile([128, 1], F32, tag="small", name="asm")
                    nc.scalar.activation(scr[:], x_sb[k][:], IDENT,
                                         accum_out=asm[:])
                    asq = small.tile([128, 1], F32, tag="small", name="asq")
                    nc.scalar.activation(scr[:], x_sb[k][:],
                                         mybir.ActivationFunctionType.Square,
                                         accum_out=asq[:])
                    nc.vector.tensor_scalar(out=stats_all[:, 0, k:k + 1],
                                            in0=asm[:], scalar1=1.0 / T,
                                            scalar2=None, op0=MUL)
                    nc.vector.tensor_scalar(out=stats_all[:, 1, k:k + 1],
                                            in0=asq[:], scalar1=1.0 / T,
                                            scalar2=None, op0=MUL)
                # phase 2: ONE batched group-combine matmul for all chunks
                psg8 = psA.tile([8, 2, NCHUNK], F32, tag="big", name="psg8")
                nc.tensor.matmul(psg8[:], g8_sb, stats_all[:],
                                 start=True, stop=True)
                gall = small.tile([8, 2, NCHUNK], F32, tag="small", name="gall")
                nc.vector.tensor_copy(gall[:], psg8[:])
                mu_a = gall[:, 0, :]          # [8, 4] group means
                ex_a = gall[:, 1, :]          # [8, 4] group E[x^2]
                mu2a = small.tile([8, NCHUNK], F32, tag="small", name="mu2a")
                nc.vector.tensor_mul(mu2a[:], mu_a, mu_a)
                ava = small.tile([8, NCHUNK], F32, tag="small", name="ava")
                nc.vector.scalar_tensor_tensor(
                    out=ava[:], in0=ex_a, scalar=EPS, in1=mu2a[:],
                    op0=ADD, op1=mybir.AluOpType.subtract)
                yia = small.tile([8, NCHUNK], I32, tag="small", name="yia")
                nc.vector.tensor_scalar(out=yia[:], in0=ava[:].bitcast(I32),
                                        scalar1=1, scalar2=None, op0=SHR)
                mga = small.tile([8, NCHUNK], I32, tag="small", name="mga")
                nc.vector.memset(mga[:], MAGIC)
                nc.vector.tensor_sub(yia[:], mga[:], yia[:])
                ya = yia[:].bitcast(F32)
                t2a = small.tile([8, NCHUNK], F32, tag="small", name="t2a")
                nc.vector.tensor_mul(t2a[:], ya, ya)
                nc.vector.tensor_mul(t2a[:], t2a[:], ava[:])
                nc.vector.tensor_scalar(out=t2a[:], in0=t2a[:], scalar1=-0.5,
                                        scalar2=1.5, op0=MUL, op1=ADD)
                grpa = small.tile([8, 2, NCHUNK], F32R, tag="small", name="grpa")
                nc.vector.tensor_copy(grpa[:, 0, :], mu_a)
                nc.vector.tensor_mul(grpa[:, 1, :], ya, t2a[:])
                psca = psA.tile([128, 2 * NCHUNK], F32, tag="big", name="psca")
                nc.tensor.matmul(psca[:], gt8_sb[:],
                                 grpa[:].rearrange("g a k -> g (a k)"),
                                 start=True, stop=True)
                s_a = small.tile([128, NCHUNK], F32, tag="small", name="s_a")
                nc.vector.tensor_mul(s_a[:], psca[:, NCHUNK:2 * NCHUNK],
                                     gnw_sb)
                t1a = small.tile([128, NCHUNK], F32, tag="small", name="t1a")
                nc.vector.tensor_mul(t1a[:], psca[:, 0:NCHUNK], s_a[:])
                b_a = small.tile([128, NCHUNK], F32, tag="small", name="b_a")
                nc.vector.tensor_sub(b_a[:], gnb_sb, t1a[:])
                # affine: nb0 halves first (unblocks the first qk tiles)
                for k in range(NCHUNK):
                    h_sb[k] = singles.tile([128, T], BF16, tag=f"h{k}",
                                           name=f"h{k}")
                for nb in range(NB):
                    sl = slice(512 * nb, 512 * (nb + 1))
                    for k in range(NCHUNK):
                        if k == 2:
                            nc.vector.tensor_scalar(
                                out=h_sb[k][:, sl], in0=x_sb[k][:, sl],
                                scalar1=s_a[:, k:k + 1],
                                scalar2=b_a[:, k:k + 1], op0=MUL, op1=ADD)
                        else:
                            nc.scalar.activation(h_sb[k][:, sl],
                                                 x_sb[k][:, sl], IDENT,
                                                 bias=b_a[:, k:k + 1],
                                                 scale=s_a[:, k:k + 1])

                # ================= qk tiles =================
                qk_tiles = {}

                def gen_qk01():
                    # m = 0, 1 interleaved nb-major so QK(0) steps on the
                    # first t-half can start as early as possible
                    pqs = [psA.tile([128, T], F32, tag="big", name="pq")
                           for _ in range(2)]
                    for m in range(2):
                        qk_tiles[m] = singles.tile(
                            [128, T], BF16, tag=f"qk{m}{sfx}", name=f"qk{m}")
                    for nb in range(NB):
                        sl = slice(512 * nb, 512 * (nb + 1))
                        for m in range(2):
                            for i, k in enumerate(KORD):
                                nc.tensor.matmul(
                                    pqs[m][:, sl], wqk_ap(k, m),
                                    h_sb[k][:, sl], start=(i == 0),
                                    stop=(i == 3))
                        nc.scalar.activation(qk_tiles[0][:, sl], pqs[0][:, sl],
                                             IDENT, bias=bqk_sb[:, 0:1])
                        nc.vector.tensor_scalar(
                            out=qk_tiles[1][:, sl], in0=pqs[1][:, sl],
                            scalar1=bqk_sb[:, 1:2], scalar2=None, op0=ADD)

                gen_qk01()

                # ================= attention state =================
                ew_pairs = {}

                def _ew(h, j):
                    if (h, j) not in ew_pairs:
                        ew_pairs[(h, j)] = ewp.tile([128, 2, T], FP8, tag="ew",
                                                    name=f"ew{h}_{j}")
                    return ew_pairs[(h, j)]

                def emit_qk_step(h, st):
                    # logits for head h, s-tile st: 2 matmuls + 1 merged exp
                    p, slot = h // 2, h % 2
                    lo, hi = CH * slot, CH * (slot + 1)
                    ktile, qtile = qk_tiles[2 * p], qk_tiles[2 * p + 1]
                    j, pl = st // 2, st % 2
                    pw = psB.tile([128, T], F32, tag="pw", name="pw")
                    for nb in range(NB):
                        nc.tensor.matmul(
                            pw[:, 512 * nb:512 * (nb + 1)],
                            ktile[lo:hi, 128 * st:128 * (st + 1)],
                            qtile[lo:hi, 512 * nb:512 * (nb + 1)],
                            start=True, stop=True)
                    nc.scalar.activation(_ew(h, j)[:, pl, :], pw[:], EXP)

                def emit_qk_half_step(h, st, nb):
                    # one t-half of head h's logits (used to stretch head 7's
                    # exp stream over the last two windows)
                    p, slot = h // 2, h % 2
                    lo, hi = CH * slot, CH * (slot + 1)
                    ktile, qtile = qk_tiles[2 * p], qk_tiles[2 * p + 1]
                    j, pl = st // 2, st % 2
                    sl = slice(512 * nb, 512 * (nb + 1))
                    pw = psB.tile([128, 512], F32, tag="pw", name="pwh")
                    nc.tensor.matmul(
                        pw[:], ktile[lo:hi, 128 * st:128 * (st + 1)],
                        qtile[lo:hi, sl], start=True, stop=True)
                    nc.scalar.activation(_ew(h, j)[:, pl, sl], pw[:], EXP)

                # vt pair tiles (fp8, rider block cols 0:RID with col0 = ones)
                vt_sb = [singles.tile([128, 2, H, RID + CH], FP8,
                                      tag=f"vt{j}", name=f"vt{j}")
                         for j in range(NJ)]
                for j in range(NJ):
                    nc.vector.memset(vt_sb[j][:, :, :, 0:RID], 1.0)

                def emit_v_tile(st):
                    pv = psA.tile([128, C], F32, tag="big", name="pv")
                    for i, k in enumerate(KORD):
                        nc.tensor.matmul(pv[:],
                                         h_sb[k][:, 128 * st:128 * (st + 1)],
                                         wv_ap(k), start=(i == 0),
                                         stop=(i == 3))
                    nc.vector.tensor_copy(
                        vt_sb[st // 2][:, st % 2, :, RID:RID + CH],
                        pv[:].rearrange("p (h c) -> p h c", h=H))

                a_sb = [singles.tile([128, T], BF16, tag=f"a{p}",
                                     name=f"a{p}{sfx}") for p in range(NCHUNK)]
                acc_sb = [singles.tile([128, T], F32, tag=f"acc{m}",
                                       name=f"acc{m}{sfx}")
                          for m in range(NCHUNK)]

                # ---- prologue: v tiles + QK(0) steps (pv uses the big tag
                # ---- so the pw rotation stays a pure QK/exp double-buffer)
                for st in range(NT):
                    if st < 2:
                        emit_qk_half_step(0, st, 0)
                        emit_qk_half_step(0, st, 1)
                    else:
                        emit_qk_step(0, st)
                    emit_v_tile(st)

                # ================= duties =================
                def qk_spread_duty(m):
                    pq = psA.tile([128, T], F32, tag="big", name="pq")
                    qk = singles.tile([128, T], BF16, tag=f"qk{m}{sfx}",
                                      name=f"qk{m}")
                    qk_tiles[m] = qk
                    for nb in range(NB):
                        sl = slice(512 * nb, 512 * (nb + 1))
                        for i, k in enumerate(KORD):
                            nc.tensor.matmul(
                                pq[:, sl], wqk_ap(k, m), h_sb[k][:, sl],
                                start=(i == 0), stop=(i == 3))
                            yield
                        nc.vector.tensor_scalar(
                            out=qk[:, sl], in0=pq[:, sl],
                            scalar1=bqk_sb[:, m:m + 1], scalar2=None, op0=ADD)
                    yield

                def proj01_duty():
                    # acc[m] = (Wp0 @ a0 + Wp1 @ a1 + bproj) + x
                    for m in range(NCHUNK):
                        po = psA.tile([128, T], F32, tag="big", name="po")
                        for nb in range(NB):
                            sl = slice(512 * nb, 512 * (nb + 1))
                            nc.tensor.matmul(
                                po[:, sl], wproj_ap(0, m), a_sb[0][:, sl],
                                start=True, stop=False)
                            yield
                            nc.tensor.matmul(
                                po[:, sl], wproj_ap(1, m), a_sb[1][:, sl],
                                start=False, stop=True)
                            yield
                        nc.vector.scalar_tensor_tensor(
                            out=acc_sb[m][:], in0=po[:],
                            scalar=bproj_sb[:, m:m + 1], in1=x_sb[m][:],
                            op0=ADD, op1=ADD)
                        yield

                def proj2_duty(mlo, mhi):
                    # acc[m] += Wp2 @ a2
                    for m in range(mlo, mhi):
                        po = psA.tile([128, T], F32, tag="big", name="po")
                        for nb in range(NB):
                            sl = slice(512 * nb, 512 * (nb + 1))
                            nc.tensor.matmul(
                                po[:, sl], wproj_ap(2, m), a_sb[2][:, sl],
                                start=True, stop=True)
                            yield
                        nc.vector.tensor_add(acc_sb[m][:], po[:], acc_sb[m][:])
                        yield

                # ================= head loop =================
                last_rep = rep == n_reps - 1

                def tail_nb(nb, pa7):
                    # normalize head 7's nb half, then pair-3 proj + out
                    sl = slice(512 * nb, 512 * (nb + 1))
                    rcb = recp.tile([CH, 512], F32, tag="rcb", name="rcb")
                    nc.vector.reciprocal_approx_fast(rcb[:], pa7[0:CH, sl])
                    nc.vector.tensor_mul(a_sb[3][CH:128, sl],
                                         pa7[RID:RID + CH, sl], rcb[:])
                    yield
                    for m in range(NCHUNK):
                        po = psA.tile([128, 512], F32, tag="big", name="pot")
                        nc.tensor.matmul(po[:], wproj_ap(3, m),
                                         a_sb[3][:, sl], start=True, stop=True)
                        nc.vector.tensor_add(acc_sb[m][:, sl], po[:],
                                             acc_sb[m][:, sl])
                        if last_rep:
                            nc.sync.dma_start(out_d[128 * m:128 * (m + 1), sl],
                                              acc_sb[m][:, sl])
                        yield

                pa7 = None
                for h in range(H - 1):
                    p, slot = h // 2, h % 2
                    lo, hi = CH * slot, CH * (slot + 1)
                    duties = []
                    if h == 0:
                        duties.append(qk_spread_duty(2))
                        duties.append(qk_spread_duty(3))
                    elif h == 1:
                        duties.append(qk_spread_duty(4))
                        duties.append(qk_spread_duty(5))
                    elif h == 2:
                        duties.append(qk_spread_duty(6))
                        duties.append(qk_spread_duty(7))
                    elif h == 4:
                        duties.append(proj01_duty())
                    elif h == 5:
                        duties.append(proj2_duty(0, 4))
                    pa = psA.tile([128, T], F32, tag="big", name="pa")
                    # feed ACT before the AV burst
                    npre = 3 if h < 3 else 2
                    if h < 6:
                        for st0 in range(npre):
                            emit_qk_step(h + 1, st0)
                    else:
                        emit_qk_half_step(7, 0, 0)
                        emit_qk_half_step(7, 1, 0)
                    # ew for this head is complete: AV burst, then normalize
                    for j in range(NJ):
                        ewt = ew_pairs.pop((h, j))
                        for nb in range(NB):
                            nc.tensor.matmul(
                                pa[:, 512 * nb:512 * (nb + 1)],
                                vt_sb[j][:, :, h, :],
                                ewt[:, :, 512 * nb:512 * (nb + 1)],
                                start=(j == 0), stop=(j == NJ - 1),
                                perf_mode=DR)
                    rec = recp.tile([CH, T], F32, tag="rec", name="rec")
                    nc.vector.reciprocal_approx_fast(rec[:], pa[0:CH, :])
                    nc.vector.tensor_mul(a_sb[p][lo:hi, :],
                                         pa[RID:RID + CH, :], rec[:])
                    if h < 6:
                        for st in range(npre, NT):
                            emit_qk_step(h + 1, st)
                            advanced = 0
                            while duties and advanced < 2:
                                try:
                                    next(duties[0])
                                    advanced += 1
                                except StopIteration:
                                    duties.pop(0)
                    else:
                        # window 6 also carries head 7: nb0 halves first, so
                        # the nb0 attention/proj tail overlaps the exp stream
                        halves = ([(st, 0) for st in range(2, NT)]
                                  + [(st, 1) for st in range(NT)])
                        pa7 = psA.tile([128, T], F32, tag="big", name="pa7")
                        for step in range(NT):
                            for _ in range(2):
                                if halves:
                                    s7, n7 = halves.pop(0)
                                    emit_qk_half_step(7, s7, n7)
                            if step == 3:
                                # all nb0 logits emitted: head-7 nb0 AV
                                for j in range(NJ):
                                    nc.tensor.matmul(
                                        pa7[:, 0:512], vt_sb[j][:, :, 7, :],
                                        ew_pairs[(7, j)][:, :, 0:512],
                                        start=(j == 0), stop=(j == NJ - 1),
                                        perf_mode=DR)
                                duties.append(tail_nb(0, pa7))
                            advanced = 0
                            while duties and advanced < 2:
                                try:
                                    next(duties[0])
                                    advanced += 1
                                except StopIteration:
                                    duties.pop(0)
                    for g in duties:
                        for _ in g:
                            pass

                # ====== tail: head-7 nb1 attention + proj + out ======
                for j in range(NJ):
                    ewt = ew_pairs.pop((7, j))
                    nc.tensor.matmul(
                        pa7[:, 512:1024], vt_sb[j][:, :, 7, :],
                        ewt[:, :, 512:1024], start=(j == 0),
                        stop=(j == NJ - 1), perf_mode=DR)
                for _ in tail_nb(1, pa7):
                    pass

    nc.compile()
    return nc


def _get_program(n_reps=1):
    key = ("prog", n_reps)
    if key not in _CACHE:
        _CACHE[key] = _build_program(n_reps)
    return _CACHE[key]


def kernel(x, gn_w, gn_b, qkv_w, qkv_b, proj_w, proj_b, _n_reps=1):
    x = np.asarray(x, dtype=np.float32)
    hw = _host_weights(np.asarray(gn_w, np.float32), np.asarray(gn_b, np.float32),
                       np.asarray(qkv_w, np.float32), np.asarray(qkv_b, np.float32),
                       np.asarray(proj_w, np.float32), np.asarray(proj_b, np.float32))
    xr = np.ascontiguousarray(x.reshape(B, C, T))
    nc = _get_program(_n_reps)
    in_maps = [dict(hw, x=xr[b]) for b in range(B)]
    res = run_bass_kernel_spmd(nc, in_maps, core_ids=list(range(B)))
    out = np.stack([res.results[b]["out"] for b in range(B)])
    return out.reshape(B, C, HS, WS).astype(np.float32)
